# revision 1
# baseline (speedup 1.0000x reference)
"""Trainium2 Bass kernel for nn_Block_31954556682442 (spiking MoE-SSA block).

Sharding: pure data-parallel over batch B=8 -> one sample (4 LIF time steps)
per NeuronCore, zero collectives. v2 design:
  - all weight matmuls as bf16 hi/lo split (3-term W@x for fc1/kq/v with
    bf16-split activations; 2-term for proj/fc2 whose rhs are exact bf16
    integers), residual error ~2^-18 -> no spike flips observed
  - time steps batched into matmul free dims (N=512 covers 2 steps)
  - bf16 exact-integer attention core (spikes are {0,1})
  - LIF scans in 2^t-scaled form: membrane update = tensor_add on GPSIMD,
    spike/reset = tensor_scalar/scalar_tensor_tensor on DVE (threshold 2^t)
  - depthwise 3x3 conv t-batched: 9 shifted per-partition-scalar MACs over
    (128, 4*256) tiles on DVE, 2^t applied at the LIF add
  - PSUM evicts fused with BN scale+bias (+2^t*0.5) on ScalarE
Self-contained: hardcodes all shapes; no sibling imports.
"""
import numpy as np
import ml_dtypes

import concourse.bacc as bacc
import concourse.mybir as mybir
import concourse.tile as tile
from concourse.bass_utils import run_bass_kernel_spmd

F32 = mybir.dt.float32
BF16 = mybir.dt.bfloat16
AL = mybir.AluOpType
AF = mybir.ActivationFunctionType

T, B, C, N = 4, 8, 384, 256
ED = 96
NE = 4
NU = 5
HID, HH = 2048, 1024
S = float(1.0 / np.sqrt(1.0 + 1e-5))
P = 128


def _body(nc, tc, d):
    from contextlib import ExitStack
    VE = nc.vector
    GE = nc.gpsimd

    with ExitStack() as ctx:
        def pool(name, bufs, space="SBUF"):
            return ctx.enter_context(tc.tile_pool(name=name, bufs=bufs, space=space))

        wp = pool("wp", 1)
        mp = pool("mp", 1)
        ps_m = pool("ps_m", 2, "PSUM")
        ps_o = pool("ps_o", 6, "PSUM")
        xs_p = pool("xs_p", 3)       # (128,1024) f32, doubles as x_new
        sphl_p = pool("sphl_p", 3)   # bf16 hi splits
        splo_p = pool("splo_p", 3)   # bf16 lo splits
        xkq_p = pool("xkq_p", 2)     # (96,1280) f32
        xev_p = pool("xev_p", 4)     # (128,768) f32 evict/LIF targets
        xrt_p = pool("xrt_p", 2)     # (128,8)
        sp_p = pool("sp_p", 4)       # (96,1280) bf16 kq spikes
        vsp_p = pool("vsp_p", 4)     # (128,768) bf16
        wsp_p = pool("wsp_p", 4)     # (128,8) f32
        at_p = pool("at_p", 3)       # (128,256) bf16
        rsp_p = pool("rsp_p", 2)     # (128,768) bf16
        y_p = pool("y_p", 8)         # (128,384) bf16
        ydn_p = pool("ydn_p", 3)     # (128,1024) bf16
        xh_p = pool("xh_p", 2)       # (128,2048) f32
        spch_p = pool("spch_p", 2)   # (128,2048) bf16
        acc_p = pool("acc_p", 2)     # (128,1024) f32
        mg_p = pool("mg_p", 2)       # (128,1024) bf16
        mh_p = pool("mh_p", 2)       # (128,512) f32
        mdw_p = pool("mdw_p", 2)     # (128,256) f32

        # ---------------- weight loads ----------------
        def wload(name, shape, dt=F32, src=None):
            w = wp.tile(shape, dt, name=name, tag=name)
            nc.sync.dma_start(out=w, in_=d[name] if src is None else src)
            return w

        ident = wload('ident', [P, P], BF16)
        # PE warmup: ~60 dummy matmuls to flip HAM to K=8/8 before phase A
        pwarm = ps_m.tile([P, P], F32, name="pwarm", tag="pm")
        for wi in range(60):
            nc.tensor.matmul(pwarm, ident, ident, start=True, stop=True)
        warm_sink = wp.tile([P, 1], F32, name="warm_sink", tag="warm_sink")
        nc.scalar.activation(warm_sink, pwarm[:, 0:1], AF.Copy)

        # xs first (A-phase starts on these)
        xs_kt = []
        for kt in range(3):
            x_ = xs_p.tile([P, 4 * N], F32, name=f"xs{kt}", tag="t")
            xs_kt.append(x_)
        for kt in range(3):
            nc.sync.dma_start(out=xs_kt[kt], in_=d['xin'][kt*P:(kt+1)*P, :])
        # combined weight tiles (hi | lo in one DMA each)
        kq2, v2, pj2, f12, f22, r_w = [], [], [], [], [], []
        for kt in range(3):
            kq2.append(wload(f'kq2_{kt}', [P, 960], BF16, d['kq_w2'][kt*P:(kt+1)*P, :]))
        skq = wload('s_kq', [96, 40])
        a_kq, b_kq = skq[:, 0:20], skq[:, 20:40]
        for kt in range(3):
            v2.append(wload(f'v2_{kt}', [P, 768], BF16, d['v_w2'][kt*P:(kt+1)*P, :]))
            r_w.append(wload(f'r_w{kt}', [P, 4], F32, d['r_wT'][kt*P:(kt+1)*P, :]))
        rb = wload('r_b', [1, 4]); ones = wload('ones', [1, P])
        for kt in range(3):
            pj2.append(wload(f'pj2_{kt}', [P, 768], BF16, d['pj_w2'][kt*P:(kt+1)*P, :]))
        spo = wload('s_po', [P, 48])
        a_p, b_p = spo[:, 0:12], spo[:, 12:24]
        a_o, b_o = spo[:, 24:36], spo[:, 36:48]
        for kt in range(3):
            f12.append(wload(f'f12_{kt}', [P, 4096], BF16, d['f1_w2'][kt*P:(kt+1)*P, :]))
        sh = wload('s_h', [P, 128])
        a_h, b_h = sh[:, 0:64], sh[:, 64:128]
        sdw = wload('s_dw', [P, 152])
        dwt, dwtn, b_dw = sdw[:, 0:72], sdw[:, 72:144], sdw[:, 144:152]
        for ch in range(8):
            f22.append(wload(f'f22_{ch}', [P, 768], BF16, d['f2_w2'][ch*P:(ch+1)*P, :]))
        kqh = [w[:, 0:480] for w in kq2]; kql = [w[:, 480:960] for w in kq2]
        vh = [w[:, 0:384] for w in v2]; vl = [w[:, 384:768] for w in v2]
        pjh = [w[:, 0:384] for w in pj2]; pjl = [w[:, 384:768] for w in pj2]
        f1h = [w[:, 0:2048] for w in f12]; f1l = [w[:, 2048:4096] for w in f12]
        f2h = [w[:, 0:384] for w in f22]; f2l = [w[:, 384:768] for w in f22]

        # ---------------- xs bf16 splits ----------------
        xhi, xlo = [], []
        for kt in range(3):
            h_ = sphl_p.tile([P, 4 * N], BF16, name=f"xhi{kt}", tag="t")
            nc.scalar.activation(h_, xs_kt[kt], AF.Copy)
            l_ = splo_p.tile([P, 4 * N], BF16, name=f"xlo{kt}", tag="t")
            GE.tensor_sub(l_, xs_kt[kt], h_)
            xhi.append(h_); xlo.append(l_)

        # ---------------- phase A: kq / v / router matmuls + evicts ----------------
        m_kq = mp.tile([96, 5 * N], F32, name="m_kq", tag="m_kq")
        m_vt = mp.tile([P, 768], F32, name="m_vt", tag="m_vt")
        m_rt = mp.tile([P, 8], F32, name="m_rt", tag="m_rt")
        m_p = mp.tile([P, 768], F32, name="m_p", tag="m_p")
        m_o = [mp.tile([P, N], F32, name=f"m_o{i}", tag=f"m_o{i}") for i in range(3)]

        xkq_t = [xkq_p.tile([96, 5 * N], F32, name=f"xkq{t}", tag="t") for t in range(T)]
        xvt_t = [xev_p.tile([P, 768], F32, name=f"xvt{t}", tag="t") for t in range(T)]
        xrt_t = [xrt_p.tile([P, 8], F32, name=f"xrt{t}", tag="t") for t in range(T)]

        for tp in range(2):
            for u in range(NU):
                pt = ps_m.tile([96, 512], F32, name=f"pkq{u}_{tp}", tag="pm")
                first = True
                for kt in range(3):
                    rh = xhi[kt][:, tp*512:(tp+1)*512]
                    rl = xlo[kt][:, tp*512:(tp+1)*512]
                    for w_, r_ in ((kqh[kt], rh), (kqh[kt], rl), (kql[kt], rh)):
                        nc.tensor.matmul(pt, w_[:, 96*u:96*(u+1)], r_,
                                         start=first, stop=(kt == 2 and r_ is rh and w_ is kql[kt]))
                        first = False
                for ti in range(2):
                    t = tp * 2 + ti
                    c = u * 4 + t
                    nc.scalar.activation(xkq_t[t][:, u*N:(u+1)*N], pt[:, ti*N:(ti+1)*N],
                                         AF.Identity, bias=b_kq[:, c:c+1], scale=a_kq[:, c:c+1])
        for t in range(T):
            for mt in range(2):
                pv = ps_m.tile([P, 384], F32, name=f"pvt{t}_{mt}", tag="pm")
                first = True
                for kt in range(3):
                    lh = xhi[kt][:, t*N + mt*P: t*N + (mt+1)*P]
                    ll = xlo[kt][:, t*N + mt*P: t*N + (mt+1)*P]
                    for l_, w_ in ((lh, vh[kt]), (ll, vh[kt]), (lh, vl[kt])):
                        nc.tensor.matmul(pv, l_, w_, start=first,
                                         stop=(kt == 2 and l_ is lh and w_ is vl[kt]))
                        first = False
                nc.scalar.activation(xvt_t[t][:, mt*384:(mt+1)*384], pv, AF.Copy,
                                     bias=0.0, scale=0.5 * float(2.0 ** t))
            for mt in range(2):
                pr = ps_m.tile([P, 4], F32, name=f"prt{t}_{mt}", tag="pm")
                for kt in range(3):
                    nc.tensor.matmul(pr, xs_kt[kt][:, t*N + mt*P: t*N + (mt+1)*P],
                                     r_w[kt], start=(kt == 0), stop=False)
                nc.tensor.matmul(pr, ones, rb, start=False, stop=True)
                nc.scalar.activation(xrt_t[t][:, mt*4:(mt+1)*4], pr, AF.Copy,
                                     bias=0.0, scale=float(2.0 ** t))

        # ---------------- phase B: LIF scans for kq / v / r ----------------
        sp_t, v_sp, w_sp = [], [], []
        for t in range(T):
            thr = float(2.0 ** t)
            U = xkq_t[t]
            if t > 0:
                GE.tensor_add(U, m_kq, U)
            sp = sp_p.tile([96, 5 * N], BF16, name=f"sp{t}", tag="t")
            VE.tensor_single_scalar(sp, U, thr, AL.is_ge)
            if t < T - 1:
                VE.scalar_tensor_tensor(out=m_kq, in0=U, scalar=thr, in1=U,
                                        op0=AL.is_lt, op1=AL.mult)
            sp_t.append(sp)

            U = xvt_t[t]
            if t > 0:
                GE.tensor_add(U, m_vt, U)
            vs = vsp_p.tile([P, 768], BF16, name=f"vsp{t}", tag="t")
            VE.tensor_single_scalar(vs, U, thr, AL.is_ge)
            if t < T - 1:
                VE.scalar_tensor_tensor(out=m_vt, in0=U, scalar=thr, in1=U,
                                        op0=AL.is_lt, op1=AL.mult)
            v_sp.append(vs)

            U = xrt_t[t]
            if t > 0:
                GE.tensor_add(U, m_rt, U)
            ws = wsp_p.tile([P, 8], F32, name=f"wsp{t}", tag="t")
            VE.tensor_single_scalar(ws, U, thr, AL.is_ge)
            if t < T - 1:
                VE.scalar_tensor_tensor(out=m_rt, in0=U, scalar=thr, in1=U,
                                        op0=AL.is_lt, op1=AL.mult)
            w_sp.append(ws)

        # ---------------- phase C: experts ----------------
        y = [[None] * 2 for _ in range(T)]
        m_res_e = [mp.tile([P, 768], F32, name=f"m_res{e}", tag=f"m_res{e}")
                   for e in range(NE)]
        for t in range(T):
            thr = float(2.0 ** t)
            for e in range(NE):
                m_res = m_res_e[e]
                at_sb = []
                for mt in range(2):
                    pa = ps_m.tile([P, N], F32, name=f"pat{e}{t}{mt}", tag="pm")
                    nc.tensor.matmul(pa, sp_t[t][:, mt*P:(mt+1)*P],
                                     sp_t[t][:, (1+e)*N:(2+e)*N], start=True, stop=True)
                    ats = at_p.tile([P, N], BF16, name=f"at{e}{t}{mt}", tag="t")
                    nc.scalar.activation(ats, pa, AF.Copy)
                    at_sb.append(ats)
                xr = xev_p.tile([P, 768], F32, name=f"xres{e}{t}", tag="t")
                for mt in range(2):
                    pr_ = ps_m.tile([P, 384], F32, name=f"pres{e}{t}{mt}", tag="pm")
                    for mk in range(2):
                        nc.tensor.matmul(pr_, at_sb[mk][:, mt*P:(mt+1)*P],
                                         v_sp[t][:, mk*384:(mk+1)*384],
                                         start=(mk == 0), stop=(mk == 1))
                    nc.scalar.activation(xr[:, mt*384:(mt+1)*384], pr_, AF.Copy,
                                         bias=0.0, scale=0.5 * thr)
                U = xr
                if t > 0:
                    GE.tensor_add(U, m_res, U)
                rs = rsp_p.tile([P, 768], BF16, name=f"rsp{e}{t}", tag="t")
                VE.tensor_single_scalar(rs, U, thr, AL.is_ge)
                if t < T - 1:
                    VE.scalar_tensor_tensor(out=m_res, in0=U, scalar=thr, in1=U,
                                            op0=AL.is_lt, op1=AL.mult)
                for mt in range(2):
                    if e == 0:
                        yt = y_p.tile([P, 384], BF16, name=f"y{t}_{mt}", tag="t")
                        VE.scalar_tensor_tensor(
                            out=yt, in0=rs[:, mt*384:(mt+1)*384],
                            scalar=w_sp[t][:, mt*4:mt*4+1],
                            in1=rs[:, mt*384:(mt+1)*384], op0=AL.mult, op1=AL.bypass)
                        y[t][mt] = yt
                    else:
                        VE.scalar_tensor_tensor(
                            out=y[t][mt], in0=rs[:, mt*384:(mt+1)*384],
                            scalar=w_sp[t][:, mt*4+e:mt*4+e+1],
                            in1=y[t][mt], op0=AL.mult, op1=AL.add)

        # ---------------- phase D: transpose y, proj, LIF, residual ----------------
        # tp-pipelined: each time-pair's transposes -> proj -> LIF -> splits
        ydn = [ydn_p.tile([P, 4 * N], BF16, name=f"ydn{dt}", tag="t") for dt in range(3)]
        xp_t = [xev_p.tile([P, 768], F32, name=f"xp{t}", tag="t") for t in range(T)]
        xnhi = [sphl_p.tile([P, 4 * N], BF16, name=f"xnhi{kt}", tag="t") for kt in range(3)]
        xnlo = [splo_p.tile([P, 4 * N], BF16, name=f"xnlo{kt}", tag="t") for kt in range(3)]
        for tp in range(2):
            for t in (tp * 2, tp * 2 + 1):
                for mt in range(2):
                    for dt in range(3):
                        ptr = ps_m.tile([P, P], BF16, name=f"ptr{t}{mt}{dt}", tag="pm")
                        nc.tensor.transpose(ptr, y[t][mt][:, dt*P:(dt+1)*P], ident)
                        nc.scalar.activation(ydn[dt][:, t*N + mt*P: t*N + (mt+1)*P],
                                             ptr, AF.Copy)
            for mt in range(3):
                pp = ps_m.tile([P, 512], F32, name=f"pp{mt}_{tp}", tag="pm")
                first = True
                for kt in range(3):
                    r_ = ydn[kt][:, tp*512:(tp+1)*512]
                    nc.tensor.matmul(pp, pjh[kt][:, mt*P:(mt+1)*P], r_,
                                     start=first, stop=False)
                    first = False
                    nc.tensor.matmul(pp, pjl[kt][:, mt*P:(mt+1)*P], r_,
                                     start=False, stop=(kt == 2))
                for ti in range(2):
                    t = tp * 2 + ti
                    c = mt * 4 + t
                    nc.scalar.activation(xp_t[t][:, mt*N:(mt+1)*N], pp[:, ti*N:(ti+1)*N],
                                         AF.Identity, bias=b_p[:, c:c+1], scale=a_p[:, c:c+1])
            for t in (tp * 2, tp * 2 + 1):
                thr = float(2.0 ** t)
                U = xp_t[t]
                if t > 0:
                    GE.tensor_add(U, m_p, U)
                if t < T - 1:
                    VE.scalar_tensor_tensor(out=m_p, in0=U, scalar=thr, in1=U,
                                            op0=AL.is_lt, op1=AL.mult)
                for mt in range(3):
                    # x_new overwrites xs in place (residual add)
                    VE.scalar_tensor_tensor(
                        out=xs_kt[mt][:, t*N:(t+1)*N], in0=U[:, mt*N:(mt+1)*N],
                        scalar=thr, in1=xs_kt[mt][:, t*N:(t+1)*N],
                        op0=AL.is_ge, op1=AL.add)
            # x_new bf16 splits for this time-pair (fc1 can start on tp=0)
            for kt in range(3):
                nc.scalar.activation(xnhi[kt][:, tp*512:(tp+1)*512],
                                     xs_kt[kt][:, tp*512:(tp+1)*512], AF.Copy)
                GE.tensor_sub(xnlo[kt][:, tp*512:(tp+1)*512],
                              xs_kt[kt][:, tp*512:(tp+1)*512],
                              xnhi[kt][:, tp*512:(tp+1)*512])

        # ---------------- phase E: MLP ----------------
        po = [[ps_o.tile([P, 512], F32, name=f"po{tp}_{mt}", tag="po")
               for mt in range(3)] for tp in range(2)]
        for ch in range(8):
            xh = xh_p.tile([P, 2048], F32, name=f"xh{ch}", tag="t")
            for half in range(2):
                mth = ch + 8 * half
                for tp in range(2):
                    ph = ps_m.tile([P, 512], F32, name=f"ph{ch}{half}{tp}", tag="pm")
                    first = True
                    for kt in range(3):
                        rh = xnhi[kt][:, tp*512:(tp+1)*512]
                        rl = xnlo[kt][:, tp*512:(tp+1)*512]
                        for w_, r_ in ((f1h[kt], rh), (f1h[kt], rl), (f1l[kt], rh)):
                            nc.tensor.matmul(ph, w_[:, mth*P:(mth+1)*P], r_,
                                             start=first,
                                             stop=(kt == 2 and r_ is rh and w_ is f1l[kt]))
                            first = False
                    for ti in range(2):
                        t = tp * 2 + ti
                        c = mth * 4 + t
                        nc.scalar.activation(
                            xh[:, half*1024 + t*N: half*1024 + (t+1)*N],
                            ph[:, ti*N:(ti+1)*N], AF.Identity,
                            bias=b_h[:, c:c+1], scale=a_h[:, c:c+1])
            # h-LIF over t (both halves via 3D APs)
            m_h = mh_p.tile([P, 512], F32, name=f"m_h{ch}", tag="t")
            sp_ch = spch_p.tile([P, 2048], BF16, name=f"spch{ch}", tag="t")
            xh3 = xh.rearrange("p (h q) -> p h q", h=2)
            mh3 = m_h.rearrange("p (h q) -> p h q", h=2)
            spc3 = sp_ch.rearrange("p (h q) -> p h q", h=2)
            for t in range(T):
                thr = float(2.0 ** t)
                U3 = xh3[:, :, t*N:(t+1)*N]
                if t > 0:
                    GE.tensor_add(U3, mh3, U3)
                VE.tensor_single_scalar(spc3[:, :, t*N:(t+1)*N], U3, thr, AL.is_ge)
                if t < T - 1:
                    VE.scalar_tensor_tensor(out=mh3, in0=U3, scalar=thr, in1=U3,
                                            op0=AL.is_lt, op1=AL.mult)
            # depthwise conv, t-batched, unscaled taps
            acc = acc_p.tile([P, 1024], F32, name=f"acc{ch}", tag="t")
            VE.tensor_scalar(acc, sp_ch[:, 0:1024], dwt[:, ch*9+4:ch*9+5],
                             b_dw[:, ch:ch+1], AL.mult, AL.add)
            x1f = sp_ch[:, 0:1024]
            x1r = x1f.rearrange("p (r w) -> p r w", w=16)     # 64 rows across t
            x1t = x1f.rearrange("p (t r) -> p t r", t=4)      # 4 t-blocks of 256
            ar = acc.rearrange("p (r w) -> p r w", w=16)
            at4 = acc.rearrange("p (t r) -> p t r", t=4)
            for dy in range(3):
                for dx in range(3):
                    if (dy, dx) == (1, 1):
                        continue
                    ct = ch * 9 + 3 * dy + dx
                    sc = dwt[:, ct:ct+1]
                    if dy == 1:
                        # pure x-shift: rows uniform across all t
                        wo0, wo1 = (1, 16) if dx == 0 else (0, 15)
                        VE.scalar_tensor_tensor(
                            out=ar[:, :, wo0:wo1], in0=x1r[:, :, wo0+dx-1:wo1+dx-1],
                            scalar=sc, in1=ar[:, :, wo0:wo1], op0=AL.mult, op1=AL.add)
                    elif dx == 1:
                        # pure y-shift: contiguous 240-element run per t-block
                        ho0 = 1 if dy == 0 else 0
                        o0 = ho0 * 16
                        i0 = o0 + (dy - 1) * 16
                        VE.scalar_tensor_tensor(
                            out=at4[:, :, o0:o0+240], in0=x1t[:, :, i0:i0+240],
                            scalar=sc, in1=at4[:, :, o0:o0+240], op0=AL.mult, op1=AL.add)
                    else:
                        # corner: full run per t-block + negated-tap edge fix
                        ho0 = 1 if dy == 0 else 0
                        o0 = ho0 * 16
                        delta = 16 * (dy - 1) + (dx - 1)
                        s_ = o0 + max(0, -(o0 + delta))
                        e_ = o0 + 240 - max(0, (o0 + delta + 240) - 256)
                        VE.scalar_tensor_tensor(
                            out=at4[:, :, s_:e_], in0=x1t[:, :, s_+delta:e_+delta],
                            scalar=sc, in1=at4[:, :, s_:e_], op0=AL.mult, op1=AL.add)
                        we = 0 if dx == 0 else 15
                        pos = [h_*16+we for h_ in range(ho0, ho0+15)
                               if s_ <= h_*16+we < e_]
                        p0, np_ = pos[0], len(pos)
                        hs, ws = (p0 + delta) // 16, (p0 + delta) % 16
                        of4 = acc.rearrange("p (t h w) -> p t h w", t=4, h=16)
                        if4 = x1f.rearrange("p (t h w) -> p t h w", t=4, h=16)
                        ofx = of4[:, :, p0//16:p0//16+np_, we:we+1].rearrange(
                            "p t h w -> p t (h w)")
                        ifx = if4[:, :, hs:hs+np_, ws:ws+1].rearrange(
                            "p t h w -> p t (h w)")
                        VE.scalar_tensor_tensor(
                            out=ofx, in0=ifx, scalar=dwtn[:, ct:ct+1],
                            in1=ofx, op0=AL.mult, op1=AL.add)
            # dw-LIF + gate -> mg (bf16)
            m_dw = mdw_p.tile([P, N], F32, name=f"m_dw{ch}", tag="t")
            mg = mg_p.tile([P, 1024], BF16, name=f"mg{ch}", tag="t")
            for t in range(T):
                thr = float(2.0 ** t)
                U = acc[:, t*N:(t+1)*N]
                if t > 0:
                    VE.scalar_tensor_tensor(out=U, in0=U, scalar=thr, in1=m_dw,
                                            op0=AL.mult, op1=AL.add)
                VE.scalar_tensor_tensor(out=mg[:, t*N:(t+1)*N], in0=U, scalar=thr,
                                        in1=sp_ch[:, 1024 + t*N: 1024 + (t+1)*N],
                                        op0=AL.is_ge, op1=AL.mult)
                if t < T - 1:
                    VE.scalar_tensor_tensor(out=m_dw, in0=U, scalar=thr, in1=U,
                                            op0=AL.is_lt, op1=AL.mult)
            # fc2 accumulate (2-term bf16, rhs exact)
            for tp in range(2):
                for mt in range(3):
                    nc.tensor.matmul(po[tp][mt], f2h[ch][:, mt*P:(mt+1)*P],
                                     mg[:, tp*512:(tp+1)*512],
                                     start=(ch == 0), stop=False, skip_group_check=True)
                    nc.tensor.matmul(po[tp][mt], f2l[ch][:, mt*P:(mt+1)*P],
                                     mg[:, tp*512:(tp+1)*512],
                                     start=False, stop=(ch == 7), skip_group_check=True)

        # fc2 evict + final LIF + residual + store
        xo_t = [xev_p.tile([P, 768], F32, name=f"xo{t}", tag="t") for t in range(T)]
        for t in range(T):
            for mt in range(3):
                c = mt * 4 + t
                nc.scalar.activation(xo_t[t][:, mt*N:(mt+1)*N],
                                     po[t // 2][mt][:, (t % 2)*N:(t % 2+1)*N],
                                     AF.Identity, bias=b_o[:, c:c+1], scale=a_o[:, c:c+1])
        for t in range(T):
            thr = float(2.0 ** t)
            for mt in range(3):
                U = xo_t[t][:, mt*N:(mt+1)*N]
                if t > 0:
                    GE.tensor_add(U, m_o[mt], U)
                if t < T - 1:
                    VE.scalar_tensor_tensor(out=m_o[mt], in0=U, scalar=thr, in1=U,
                                            op0=AL.is_lt, op1=AL.mult)
                # final out in place over xo (reset already consumed U)
                VE.scalar_tensor_tensor(
                    out=U, in0=U, scalar=thr,
                    in1=xs_kt[mt][:, t*N:(t+1)*N], op0=AL.is_ge, op1=AL.add)
                nc.sync.dma_start(out=d['out'][t*C + mt*P: t*C + (mt+1)*P, :],
                                  in_=U)


def _build():
    nc = bacc.Bacc()
    with tile.TileContext(nc) as tc:
        with tc.tile_pool(name="dram", bufs=1, space="DRAM") as dram:
            def din(name, shape, dt=F32):
                return dram.tile(shape, dt, kind="ExternalInput", name=name,
                                 uniquify=False)
            d = {
                'xin': din('xin', [C, 4 * N]),
                'out': dram.tile([T * C, N], F32, kind="ExternalOutput",
                                 name='out', uniquify=False),
                'kq_w2': din('kq_w2', [384, 960], BF16),
                's_kq': din('s_kq', [96, 40]),
                'v_w2': din('v_w2', [384, 768], BF16),
                'r_wT': din('r_wT', [384, 4]),
                'r_b': din('r_b', [1, 4]),
                'ones': din('ones', [1, 128]),
                'pj_w2': din('pj_w2', [384, 768], BF16),
                's_po': din('s_po', [128, 48]),
                'f1_w2': din('f1_w2', [384, 4096], BF16),
                's_h': din('s_h', [128, 128]),
                's_dw': din('s_dw', [128, 152]),
                'f2_w2': din('f2_w2', [1024, 768], BF16),
                'ident': din('ident', [128, 128], BF16),
            }
            _body(nc, tc, d)
    nc.finalize()
    return nc


_NC_CACHE = {}


def _get_nc():
    if 'nc' not in _NC_CACHE:
        _NC_CACHE['nc'] = _build()
    return _NC_CACHE['nc']


def _tcols(a):
    rows, k = a.shape
    out = np.empty((rows, k * 4), np.float32)
    for u in range(k):
        for t in range(4):
            out[:, u * 4 + t] = a[:, u] * (2.0 ** t)
    return out


def _split(w):
    hi = w.astype(ml_dtypes.bfloat16)
    lo = (w - hi.astype(np.float32)).astype(ml_dtypes.bfloat16)
    return hi, lo


def _prep_common(inputs):
    inp = {k: np.asarray(v, np.float32) for k, v in inputs.items()}
    k_wT = inp['k_w'].T
    exp_wT = np.concatenate([inp['exp_w'][e].T for e in range(NE)], axis=1)
    kq_wT = np.concatenate([k_wT, exp_wT], axis=1)
    a_kq = np.zeros((96, 5), np.float32)
    b_kq = np.zeros((96, 5), np.float32)
    a_kq[:, 0] = 0.5
    for e in range(NE):
        a_kq[:, 1 + e] = 0.5 * inp['exp_g'][e] * S
        b_kq[:, 1 + e] = 0.5 * inp['exp_b'][e]
    taps = inp['dw_w'][:, 0] * (0.5 * inp['dw_g'] * S)[:, None, None]
    tap2 = taps.reshape(8, 128, 9).transpose(1, 0, 2).reshape(128, 72)

    def cat2(w):
        hi, lo = _split(w)
        return np.concatenate([hi, lo], axis=1)

    com = {
        'kq_w2': cat2(kq_wT),
        's_kq': np.concatenate([_tcols(a_kq), _tcols(b_kq)], axis=1),
        'v_w2': cat2(inp['v_w'].T),
        'r_wT': inp['router_w'].T * (inp['router_g'] * S * 0.5)[None, :],
        'r_b': (0.5 * (inp['router_b'] * inp['router_g'] * S
                       + inp['router_be'])).reshape(1, 4),
        'ones': np.ones((1, 128), np.float32),
        'pj_w2': cat2(inp['proj_w'].T),
        's_po': np.concatenate([
            _tcols((0.5 * inp['proj_g'] * S).reshape(3, 128).T),
            _tcols((0.5 * (inp['proj_b'] * inp['proj_g'] * S
                           + inp['proj_be'])).reshape(3, 128).T),
            _tcols((0.5 * inp['fc2_g'] * S).reshape(3, 128).T),
            _tcols((0.5 * (inp['fc2_b'] * inp['fc2_g'] * S
                           + inp['fc2_be'])).reshape(3, 128).T)], axis=1),
        'f1_w2': cat2(inp['fc1_w'].T),
        's_h': np.concatenate([
            _tcols((0.5 * inp['fc1_g'] * S).reshape(16, 128).T),
            _tcols((0.5 * (inp['fc1_b'] * inp['fc1_g'] * S
                           + inp['fc1_be'])).reshape(16, 128).T)], axis=1),
        's_dw': np.concatenate([
            tap2, -tap2,
            (0.5 * (inp['dw_b'] * inp['dw_g'] * S
                    + inp['dw_be'])).reshape(8, 128).T], axis=1),
        'f2_w2': cat2(inp['fc2_w'].T),
        'ident': np.eye(128, dtype=ml_dtypes.bfloat16),
    }
    return {k: np.ascontiguousarray(v) for k, v in com.items()}


def run(inputs, trace=False, tmpdir=None):
    com = _prep_common(inputs)
    x = np.asarray(inputs['x'], np.float32).reshape(T, B, C, N)
    in_maps = []
    for b in range(B):
        m = dict(com)
        m['xin'] = np.ascontiguousarray(x[:, b].transpose(1, 0, 2).reshape(C, T * N))
        in_maps.append(m)
    res = run_bass_kernel_spmd(_get_nc(), in_maps, list(range(B)),
                               trace=trace, tmpdir=tmpdir)
    out = np.empty((T, B, C, N), np.float32)
    for b in range(B):
        out[:, b] = res.results[b]['out'].reshape(T, C, N)
    return out.reshape(T * B, C, 16, 16), res.exec_time_ns


def kernel(**inputs):
    out, _ = run(inputs)
    return out



# revision 8
# speedup vs baseline: 1.4105x; 1.4105x over previous
"""Trainium2 Bass kernel for nn_Block_31954556682442 (spiking MoE-SSA block).

Sharding: pure data-parallel over batch B=8 -> one sample (4 LIF time steps)
per NeuronCore, zero collectives. v3 design:
  - ALL weight matmuls single-term bf16 (W and x both bf16-rounded); CPU-sim
    rel err 4.4e-3 vs the 2e-2 gate
  - kv-first attention: res_e = q_e^T (k v^T); kv integers <=256 exact bf16;
    k^T computed in v's layout by widening the v matmul rhs (no transposes)
  - attention-path LIF state kept in bf16 (values quantized to 0.5 and
    compare-safe), enabling 2x/4x DVE modes
  - depthwise 3x3 conv moved to the PE: per-(ch,tap) diagonal bf16 weights
    x zero-padded spike tiles; bias via diag @ ones
  - fc2 split: t-pair 0 accumulated across the ch loop (3 PSUM banks),
    t-pair 1 as a tail overlapped with the final LIF
  - LIF scans in 2^t-scaled form as in v2
Self-contained: hardcodes all shapes; no sibling imports.
"""
import numpy as np
import ml_dtypes

import concourse.bacc as bacc
import concourse.mybir as mybir
import concourse.tile as tile
from concourse.bass_utils import run_bass_kernel_spmd

F32 = mybir.dt.float32
BF16 = mybir.dt.bfloat16
AL = mybir.AluOpType
AF = mybir.ActivationFunctionType

T, B, C, N = 4, 8, 384, 256
ED = 96
NE = 4
HID, HH = 2048, 1024
S = float(1.0 / np.sqrt(1.0 + 1e-5))
P = 128


def _body(nc, tc, d):
    from contextlib import ExitStack
    VE = nc.vector
    GE = nc.gpsimd

    with ExitStack() as ctx:
        def pool(name, bufs, space="SBUF"):
            return ctx.enter_context(tc.tile_pool(name=name, bufs=bufs, space=space))

        wp = pool("wp", 1)
        mp = pool("mp", 1)
        ps_m = pool("ps_m", 2, "PSUM")
        ps_dw = pool("ps_dw", 3, "PSUM")
        ps_po = pool("ps_po", 3, "PSUM")
        xs_p = pool("xs_p", 3)       # (128,1024) f32 x, doubles as x_new
        xq_p = pool("xq_p", 3)       # (128,1024) bf16 rounded x
        xqt_p = pool("xqt_p", 3)     # (96,1024) f32 q pre-act
        xvt_p = pool("xvt_p", 3)     # (128,960) f32 v|k pre-act
        xrt_p = pool("xrt_p", 2)     # (128,8)
        spq_p = pool("spq_p", 4)     # (96,1024) bf16 q spikes
        vks_p = pool("vks_p", 4)     # (128,960) bf16 v|k spikes
        wsp_p = pool("wsp_p", 4)     # (128,8) f32 router spikes
        kv_p = pool("kv_p", 2)       # (96,384) bf16
        xr_p = pool("xr_p", 3)       # (128,768) bf16 res pre-act
        rs_p = pool("rs_p", 2)       # (128,768) bf16 res spikes
        y_p = pool("y_p", 8)         # (128,384) bf16
        ydn_p = pool("ydn_p", 3)     # (128,1024) bf16
        xp_p = pool("xp_p", 2)       # (128,768) f32 proj pre-act
        xh_p = pool("xh_p", 2)       # (128,2048) f32
        sp2_p = pool("sp2_p", 2)     # (128,1024) bf16 gate spikes
        xp1_p = pool("xp1_p", 1)     # padded dw spikes, 2 tiles
        acc_p = pool("acc_p", 2)     # (128,1024) f32 dw-conv acc
        mg0_p = pool("mg0_p", 2)     # (128,512) bf16 t-pair0 gated spikes
        mg1_p = pool("mg1_p", 8)     # (128,512) bf16 t-pair1 gated spikes
        mh_p = pool("mh_p", 2)       # (128,512) f32
        mdw_p = pool("mdw_p", 1)     # (128,256) f32
        xo_p = pool("xo_p", 2)       # (128,768) f32

        # ---------------- weight loads ----------------
        def wload(name, shape, dt=F32, src=None):
            w = wp.tile(shape, dt, name=name, tag=name)
            nc.sync.dma_start(out=w, in_=d[name] if src is None else src)
            return w

        ident = wload('ident', [P, P], BF16)
        # PE warmup: dummy matmuls to flip HAM to K=8/8 before phase A
        pwarm = ps_m.tile([P, P], F32, name="pwarm", tag="pm")
        for wi in range(40):
            nc.tensor.matmul(pwarm, ident, ident, start=True, stop=True)
        warm_sink = wp.tile([P, 1], F32, name="warm_sink", tag="warm_sink")
        nc.scalar.activation(warm_sink, pwarm[:, 0:1], AF.Copy)

        # xs first (phase A starts on these)
        xs_kt = []
        for kt in range(3):
            x_ = xs_p.tile([P, 4 * N], F32, name=f"xs{kt}", tag="t")
            xs_kt.append(x_)
        for kt in range(3):
            nc.sync.dma_start(out=xs_kt[kt], in_=d['xin'][kt*P:(kt+1)*P, :])
        q_w, vk_w, pj_w, f1_w, f2_w, r_w = [], [], [], [], [], []
        for kt in range(3):
            q_w.append(wload(f'q_w{kt}', [P, 384], BF16, d['q_w'][kt*P:(kt+1)*P, :]))
            vk_w.append(wload(f'vk_w{kt}', [P, 480], BF16, d['vk_w'][kt*P:(kt+1)*P, :]))
            r_w.append(wload(f'r_w{kt}', [P, 4], F32, d['r_wT'][kt*P:(kt+1)*P, :]))
        sq = wload('s_q', [ED, 32])
        a_q, b_q = sq[:, 0:16], sq[:, 16:32]
        rb = wload('r_b', [1, 4]); ones = wload('ones', [1, P])
        for kt in range(3):
            pj_w.append(wload(f'pj_w{kt}', [P, 384], BF16, d['pj_w'][kt*P:(kt+1)*P, :]))
        spo = wload('s_po', [P, 48])
        a_p, b_p = spo[:, 0:12], spo[:, 12:24]
        a_o, b_o = spo[:, 24:36], spo[:, 36:48]
        for kt in range(3):
            f1_w.append(wload(f'f1_w{kt}', [P, 2048], BF16, d['f1_w'][kt*P:(kt+1)*P, :]))
        sh = wload('s_h', [P, 128])
        a_h, b_h = sh[:, 0:64], sh[:, 64:128]
        dwd = wload('dwd', [P, 80 * P], BF16)
        ones_b = wload('ones_b', [P, 512], BF16)
        for ch in range(8):
            f2_w.append(wload(f'f2_w{ch}', [P, 384], BF16, d['f2_w'][ch*P:(ch+1)*P, :]))

        # ---------------- x -> bf16 ----------------
        xq_kt = []
        for kt in range(3):
            q_ = xq_p.tile([P, 4 * N], BF16, name=f"xq{kt}", tag="t")
            nc.scalar.activation(q_, xs_kt[kt], AF.Copy)
            xq_kt.append(q_)

        # ---------------- phase A: q / v|k / router matmuls + evicts ----------------
        m_kq = mp.tile([ED, 4 * N], F32, name="m_kq", tag="m_kq")
        m_vt = mp.tile([P, 960], F32, name="m_vt", tag="m_vt")
        m_rt = mp.tile([P, 8], F32, name="m_rt", tag="m_rt")
        m_p = mp.tile([P, 768], F32, name="m_p", tag="m_p")
        m_o = [mp.tile([P, N], F32, name=f"m_o{i}", tag=f"m_o{i}") for i in range(3)]

        xq_t = [xqt_p.tile([ED, 4 * N], F32, name=f"xqt{t}", tag="t") for t in range(T)]
        xvt_t = [xvt_p.tile([P, 960], F32, name=f"xvt{t}", tag="t") for t in range(T)]
        xrt_t = [xrt_p.tile([P, 8], F32, name=f"xrt{t}", tag="t") for t in range(T)]

        for tp in range(2):
            for u in range(NE):
                pt = ps_m.tile([ED, 512], F32, name=f"pq{u}_{tp}", tag="pm")
                for kt in range(3):
                    nc.tensor.matmul(pt, q_w[kt][:, 96*u:96*(u+1)],
                                     xq_kt[kt][:, tp*512:(tp+1)*512],
                                     start=(kt == 0), stop=(kt == 2))
                for ti in range(2):
                    t = tp * 2 + ti
                    c = u * 4 + t
                    nc.scalar.activation(xq_t[t][:, u*N:(u+1)*N], pt[:, ti*N:(ti+1)*N],
                                         AF.Identity, bias=b_q[:, c:c+1], scale=a_q[:, c:c+1])
        for t in range(T):
            for mt in range(2):
                pv = ps_m.tile([P, 480], F32, name=f"pvt{t}_{mt}", tag="pm")
                for kt in range(3):
                    nc.tensor.matmul(pv, xq_kt[kt][:, t*N + mt*P: t*N + (mt+1)*P],
                                     vk_w[kt], start=(kt == 0), stop=(kt == 2))
                nc.scalar.activation(xvt_t[t][:, mt*480:(mt+1)*480], pv, AF.Copy,
                                     bias=0.0, scale=0.5 * float(2.0 ** t))
            for mt in range(2):
                pr = ps_m.tile([P, 4], F32, name=f"prt{t}_{mt}", tag="pm")
                for kt in range(3):
                    nc.tensor.matmul(pr, xs_kt[kt][:, t*N + mt*P: t*N + (mt+1)*P],
                                     r_w[kt], start=(kt == 0), stop=False)
                nc.tensor.matmul(pr, ones, rb, start=False, stop=True)
                nc.scalar.activation(xrt_t[t][:, mt*4:(mt+1)*4], pr, AF.Copy,
                                     bias=0.0, scale=float(2.0 ** t))

        # ---------------- phase B: LIF scans for q / v|k / r ----------------
        sp_q, vk_sp, w_sp = [], [], []
        for t in range(T):
            thr = float(2.0 ** t)
            U = xq_t[t]
            if t > 0:
                GE.tensor_add(U, m_kq, U)
            sp = spq_p.tile([ED, 4 * N], BF16, name=f"spq{t}", tag="t")
            VE.tensor_single_scalar(sp, U, thr, AL.is_ge)
            if t < T - 1:
                VE.scalar_tensor_tensor(out=m_kq, in0=U, scalar=thr, in1=U,
                                        op0=AL.is_lt, op1=AL.mult)
            sp_q.append(sp)

            U = xvt_t[t]
            if t > 0:
                GE.tensor_add(U, m_vt, U)
            vs = vks_p.tile([P, 960], BF16, name=f"vks{t}", tag="t")
            VE.tensor_single_scalar(vs, U, thr, AL.is_ge)
            if t < T - 1:
                VE.scalar_tensor_tensor(out=m_vt, in0=U, scalar=thr, in1=U,
                                        op0=AL.is_lt, op1=AL.mult)
            vk_sp.append(vs)

            U = xrt_t[t]
            if t > 0:
                GE.tensor_add(U, m_rt, U)
            ws = wsp_p.tile([P, 8], F32, name=f"wsp{t}", tag="t")
            VE.tensor_single_scalar(ws, U, thr, AL.is_ge)
            if t < T - 1:
                VE.scalar_tensor_tensor(out=m_rt, in0=U, scalar=thr, in1=U,
                                        op0=AL.is_lt, op1=AL.mult)
            w_sp.append(ws)

        # ---------------- phase C: kv-first experts ----------------
        y = [[None] * 2 for _ in range(T)]
        m_res_e = [mp.tile([P, 768], BF16, name=f"m_res{e}", tag=f"m_res{e}")
                   for e in range(NE)]
        for t in range(T):
            thr = float(2.0 ** t)
            pkv = ps_m.tile([ED, 384], F32, name=f"pkv{t}", tag="pm")
            for mt in range(2):
                nc.tensor.matmul(pkv, vk_sp[t][:, mt*480+384:(mt+1)*480],
                                 vk_sp[t][:, mt*480:mt*480+384],
                                 start=(mt == 0), stop=(mt == 1))
            kv = kv_p.tile([ED, 384], BF16, name=f"kv{t}", tag="t")
            nc.scalar.activation(kv, pkv, AF.Copy)
            for e in range(NE):
                m_res = m_res_e[e]
                xr = xr_p.tile([P, 768], BF16, name=f"xres{e}{t}", tag="t")
                for mt in range(2):
                    pr_ = ps_m.tile([P, 384], F32, name=f"pres{e}{t}{mt}", tag="pm")
                    nc.tensor.matmul(pr_, sp_q[t][:, e*N + mt*P: e*N + (mt+1)*P],
                                     kv, start=True, stop=True)
                    nc.scalar.activation(xr[:, mt*384:(mt+1)*384], pr_, AF.Copy,
                                         bias=0.0, scale=0.5 * thr)
                U = xr
                if t > 0:
                    GE.tensor_add(U, m_res, U)
                rs = rs_p.tile([P, 768], BF16, name=f"rsp{e}{t}", tag="t")
                VE.tensor_single_scalar(rs, U, thr, AL.is_ge)
                if t < T - 1:
                    VE.scalar_tensor_tensor(out=m_res, in0=U, scalar=thr, in1=U,
                                            op0=AL.is_lt, op1=AL.mult)
                for mt in range(2):
                    if e == 0:
                        yt = y_p.tile([P, 384], BF16, name=f"y{t}_{mt}", tag="t")
                        VE.scalar_tensor_tensor(
                            out=yt, in0=rs[:, mt*384:(mt+1)*384],
                            scalar=w_sp[t][:, mt*4:mt*4+1],
                            in1=rs[:, mt*384:(mt+1)*384], op0=AL.mult, op1=AL.bypass)
                        y[t][mt] = yt
                    else:
                        VE.scalar_tensor_tensor(
                            out=y[t][mt], in0=rs[:, mt*384:(mt+1)*384],
                            scalar=w_sp[t][:, mt*4+e:mt*4+e+1],
                            in1=y[t][mt], op0=AL.mult, op1=AL.add)

        # ---------------- phase D: transpose y, proj, LIF, residual ----------------
        ydn = [ydn_p.tile([P, 4 * N], BF16, name=f"ydn{dt}", tag="t") for dt in range(3)]
        xp_t = [xp_p.tile([P, 768], F32, name=f"xp{t}", tag="t") for t in range(T)]
        for tp in range(2):
            for t in (tp * 2, tp * 2 + 1):
                for mt in range(2):
                    for dt in range(3):
                        ptr = ps_m.tile([P, P], BF16, name=f"ptr{t}{mt}{dt}", tag="pm")
                        nc.tensor.transpose(ptr, y[t][mt][:, dt*P:(dt+1)*P], ident)
                        nc.scalar.activation(ydn[dt][:, t*N + mt*P: t*N + (mt+1)*P],
                                             ptr, AF.Copy)
            for mt in range(3):
                pp = ps_m.tile([P, 512], F32, name=f"pp{mt}_{tp}", tag="pm")
                for kt in range(3):
                    nc.tensor.matmul(pp, pj_w[kt][:, mt*P:(mt+1)*P],
                                     ydn[kt][:, tp*512:(tp+1)*512],
                                     start=(kt == 0), stop=(kt == 2))
                for ti in range(2):
                    t = tp * 2 + ti
                    c = mt * 4 + t
                    nc.scalar.activation(xp_t[t][:, mt*N:(mt+1)*N], pp[:, ti*N:(ti+1)*N],
                                         AF.Identity, bias=b_p[:, c:c+1], scale=a_p[:, c:c+1])
            for t in (tp * 2, tp * 2 + 1):
                thr = float(2.0 ** t)
                U = xp_t[t]
                if t > 0:
                    GE.tensor_add(U, m_p, U)
                if t < T - 1:
                    VE.scalar_tensor_tensor(out=m_p, in0=U, scalar=thr, in1=U,
                                            op0=AL.is_lt, op1=AL.mult)
                for mt in range(3):
                    # x_new overwrites xs in place (residual add)
                    VE.scalar_tensor_tensor(
                        out=xs_kt[mt][:, t*N:(t+1)*N], in0=U[:, mt*N:(mt+1)*N],
                        scalar=thr, in1=xs_kt[mt][:, t*N:(t+1)*N],
                        op0=AL.is_ge, op1=AL.add)
            # x_new -> bf16 for this time-pair (fc1 can start on tp=0)
            for kt in range(3):
                nc.scalar.activation(xq_kt[kt][:, tp*512:(tp+1)*512],
                                     xs_kt[kt][:, tp*512:(tp+1)*512], AF.Copy)

        # ---------------- phase E: MLP ----------------
        # padded dw-spike tiles (zero borders written once)
        xp1_bufs = [xp1_p.tile([P, 1296], BF16, name=f"xp1_{i}", tag=f"xp1_{i}")
                    for i in range(2)]
        for b_ in xp1_bufs:
            GE.memset(b_, 0.0)
        po1 = [ps_po.tile([P, 512], F32, name=f"po1_{mt}", tag="po")
               for mt in range(3)]
        TAPS = [(dy, dx) for dy in range(3) for dx in range(3)]
        mg_t = []
        for ch in range(8):
            xh = xh_p.tile([P, 2048], F32, name=f"xh{ch}", tag="t")
            for half in range(2):
                mth = ch + 8 * half
                for tp in range(2):
                    ph = ps_m.tile([P, 512], F32, name=f"ph{ch}{half}{tp}", tag="pm")
                    for kt in range(3):
                        nc.tensor.matmul(ph, f1_w[kt][:, mth*P:(mth+1)*P],
                                         xq_kt[kt][:, tp*512:(tp+1)*512],
                                         start=(kt == 0), stop=(kt == 2))
                    for ti in range(2):
                        t = tp * 2 + ti
                        c = mth * 4 + t
                        nc.scalar.activation(
                            xh[:, half*1024 + t*N: half*1024 + (t+1)*N],
                            ph[:, ti*N:(ti+1)*N], AF.Identity,
                            bias=b_h[:, c:c+1], scale=a_h[:, c:c+1])
            # h-LIF over t; dw-half spikes written into padded tile, gate into sp2
            m_h = mh_p.tile([P, 512], F32, name=f"m_h{ch}", tag="t")
            sp2 = sp2_p.tile([P, 1024], BF16, name=f"sp2_{ch}", tag="t")
            xp1 = xp1_bufs[ch % 2]
            xh3 = xh.rearrange("p (h q) -> p h q", h=2)
            mh3 = m_h.rearrange("p (h q) -> p h q", h=2)
            xh4 = xh.rearrange("p (a h w) -> p a h w", h=16, w=16)
            xp4 = xp1.rearrange("p (t h w) -> p t h w", t=4, h=18, w=18)
            for t in range(T):
                thr = float(2.0 ** t)
                U3 = xh3[:, :, t*N:(t+1)*N]
                if t > 0:
                    GE.tensor_add(U3, mh3, U3)
                VE.tensor_single_scalar(xp4[:, t, 1:17, 1:17], xh4[:, t], thr, AL.is_ge)
                VE.tensor_single_scalar(sp2[:, t*N:(t+1)*N],
                                        xh[:, 1024 + t*N: 1024 + (t+1)*N], thr, AL.is_ge)
                if t < T - 1:
                    VE.scalar_tensor_tensor(out=mh3, in0=U3, scalar=thr, in1=U3,
                                            op0=AL.is_lt, op1=AL.mult)
            # depthwise conv on the PE: diag(bias) @ ones + sum diag(tap) @ shifted
            acc = acc_p.tile([P, 1024], F32, name=f"acc{ch}", tag="t")
            for tb in range(2):
                pa = ps_dw.tile([P, 512], F32, name=f"dwa{ch}{tb}", tag="pdw")
                nc.tensor.matmul(pa, dwd[:, (ch*10+9)*P:(ch*10+10)*P], ones_b,
                                 start=True, stop=False)
                for j, (dy, dx) in enumerate(TAPS):
                    rhs = xp4[:, tb*2:(tb+1)*2, dy:dy+16, dx:dx+16]
                    nc.tensor.matmul(pa, dwd[:, (ch*10+j)*P:(ch*10+j+1)*P], rhs,
                                     start=False, stop=(j == 8))
                nc.scalar.activation(acc[:, tb*512:(tb+1)*512], pa, AF.Copy)
            # dw-LIF + gate -> mg0 (t-pair 0) / mg1 (t-pair 1), bf16
            m_dw = mdw_p.tile([P, N], F32, name=f"m_dw{ch}", tag="t")
            mg0 = mg0_p.tile([P, 512], BF16, name=f"mg0_{ch}", tag="t")
            mg1 = mg1_p.tile([P, 512], BF16, name=f"mg1_{ch}", tag="t")
            for t in range(T):
                thr = float(2.0 ** t)
                mg = mg0 if t < 2 else mg1
                U = acc[:, t*N:(t+1)*N]
                if t > 0:
                    VE.scalar_tensor_tensor(out=U, in0=U, scalar=thr, in1=m_dw,
                                            op0=AL.mult, op1=AL.add)
                VE.scalar_tensor_tensor(out=mg[:, (t % 2)*N:(t % 2 + 1)*N], in0=U,
                                        scalar=thr, in1=sp2[:, t*N:(t+1)*N],
                                        op0=AL.is_ge, op1=AL.mult)
                if t < T - 1:
                    VE.scalar_tensor_tensor(out=m_dw, in0=U, scalar=thr, in1=U,
                                            op0=AL.is_lt, op1=AL.mult)
            mg_t.append(mg1)
            # fc2 t-pair 0 accumulate
            for mt in range(3):
                nc.tensor.matmul(po1[mt], f2_w[ch][:, mt*P:(mt+1)*P],
                                 mg0,
                                 start=(ch == 0), stop=(ch == 7), skip_group_check=True)

        # fc2 t-pair 1 tail
        po2 = [ps_po.tile([P, 512], F32, name=f"po2_{mt}", tag="po")
               for mt in range(3)]
        for mt in range(3):
            for ch in range(8):
                nc.tensor.matmul(po2[mt], f2_w[ch][:, mt*P:(mt+1)*P],
                                 mg_t[ch],
                                 start=(ch == 0), stop=(ch == 7), skip_group_check=True)

        # fc2 evict + final LIF + residual + store
        xo_t = [xo_p.tile([P, 768], F32, name=f"xo{t}", tag="t") for t in range(T)]
        for t in range(T):
            po = po1 if t < 2 else po2
            for mt in range(3):
                c = mt * 4 + t
                nc.scalar.activation(xo_t[t][:, mt*N:(mt+1)*N],
                                     po[mt][:, (t % 2)*N:(t % 2 + 1)*N],
                                     AF.Identity, bias=b_o[:, c:c+1], scale=a_o[:, c:c+1])
        for t in range(T):
            thr = float(2.0 ** t)
            for mt in range(3):
                U = xo_t[t][:, mt*N:(mt+1)*N]
                if t > 0:
                    GE.tensor_add(U, m_o[mt], U)
                if t < T - 1:
                    VE.scalar_tensor_tensor(out=m_o[mt], in0=U, scalar=thr, in1=U,
                                            op0=AL.is_lt, op1=AL.mult)
                # final out in place over xo (reset already consumed U)
                VE.scalar_tensor_tensor(
                    out=U, in0=U, scalar=thr,
                    in1=xs_kt[mt][:, t*N:(t+1)*N], op0=AL.is_ge, op1=AL.add)
                nc.sync.dma_start(out=d['out'][t*C + mt*P: t*C + (mt+1)*P, :],
                                  in_=U)


def _build():
    nc = bacc.Bacc()
    with tile.TileContext(nc) as tc:
        with tc.tile_pool(name="dram", bufs=1, space="DRAM") as dram:
            def din(name, shape, dt=F32):
                return dram.tile(shape, dt, kind="ExternalInput", name=name,
                                 uniquify=False)
            d = {
                'xin': din('xin', [C, 4 * N]),
                'out': dram.tile([T * C, N], F32, kind="ExternalOutput",
                                 name='out', uniquify=False),
                'q_w': din('q_w', [384, 384], BF16),
                's_q': din('s_q', [ED, 32]),
                'vk_w': din('vk_w', [384, 480], BF16),
                'r_wT': din('r_wT', [384, 4]),
                'r_b': din('r_b', [1, 4]),
                'ones': din('ones', [1, 128]),
                'pj_w': din('pj_w', [384, 384], BF16),
                's_po': din('s_po', [128, 48]),
                'f1_w': din('f1_w', [384, 2048], BF16),
                's_h': din('s_h', [128, 128]),
                'dwd': din('dwd', [128, 80 * 128], BF16),
                'ones_b': din('ones_b', [128, 512], BF16),
                'f2_w': din('f2_w', [1024, 384], BF16),
                'ident': din('ident', [128, 128], BF16),
            }
            _body(nc, tc, d)
    nc.finalize()
    return nc


_NC_CACHE = {}


def _get_nc():
    if 'nc' not in _NC_CACHE:
        _NC_CACHE['nc'] = _build()
    return _NC_CACHE['nc']


def _tcols(a):
    rows, k = a.shape
    out = np.empty((rows, k * 4), np.float32)
    for u in range(k):
        for t in range(4):
            out[:, u * 4 + t] = a[:, u] * (2.0 ** t)
    return out


def _prep_common(inputs):
    inp = {k: np.asarray(v, np.float32) for k, v in inputs.items()}
    bf16 = ml_dtypes.bfloat16

    exp_wT = np.concatenate([inp['exp_w'][e].T for e in range(NE)], axis=1)
    a_q = np.zeros((ED, NE), np.float32)
    b_q = np.zeros((ED, NE), np.float32)
    for e in range(NE):
        a_q[:, e] = 0.5 * inp['exp_g'][e] * S
        b_q[:, e] = 0.5 * inp['exp_b'][e]
    vk = np.concatenate([inp['v_w'].T, inp['k_w'].T], axis=1)

    g = inp['dw_g']
    taps = (inp['dw_w'][:, 0] * (0.5 * g * S)[:, None, None]).reshape(HH, 9)
    bias = 0.5 * (inp['dw_b'] * g * S + inp['dw_be'])
    dwd = np.zeros((P, 80, P), np.float32)
    pi = np.arange(P)
    for ch in range(8):
        cg = ch * P + pi
        for j in range(9):
            dwd[pi, ch*10+j, pi] = taps[cg, j]
        dwd[pi, ch*10+9, pi] = bias[cg]

    com = {
        'q_w': exp_wT.astype(bf16),
        's_q': np.concatenate([_tcols(a_q), _tcols(b_q)], axis=1),
        'vk_w': vk.astype(bf16),
        'r_wT': inp['router_w'].T * (inp['router_g'] * S * 0.5)[None, :],
        'r_b': (0.5 * (inp['router_b'] * inp['router_g'] * S
                       + inp['router_be'])).reshape(1, 4),
        'ones': np.ones((1, 128), np.float32),
        'pj_w': inp['proj_w'].T.astype(bf16),
        's_po': np.concatenate([
            _tcols((0.5 * inp['proj_g'] * S).reshape(3, 128).T),
            _tcols((0.5 * (inp['proj_b'] * inp['proj_g'] * S
                           + inp['proj_be'])).reshape(3, 128).T),
            _tcols((0.5 * inp['fc2_g'] * S).reshape(3, 128).T),
            _tcols((0.5 * (inp['fc2_b'] * inp['fc2_g'] * S
                           + inp['fc2_be'])).reshape(3, 128).T)], axis=1),
        'f1_w': inp['fc1_w'].T.astype(bf16),
        's_h': np.concatenate([
            _tcols((0.5 * inp['fc1_g'] * S).reshape(16, 128).T),
            _tcols((0.5 * (inp['fc1_b'] * inp['fc1_g'] * S
                           + inp['fc1_be'])).reshape(16, 128).T)], axis=1),
        'dwd': dwd.reshape(P, 80 * P).astype(bf16),
        'ones_b': np.ones((P, 512), bf16),
        'f2_w': inp['fc2_w'].T.astype(bf16),
        'ident': np.eye(128, dtype=bf16),
    }
    return {k: np.ascontiguousarray(v) for k, v in com.items()}


def run(inputs, trace=False, tmpdir=None):
    com = _prep_common(inputs)
    x = np.asarray(inputs['x'], np.float32).reshape(T, B, C, N)
    in_maps = []
    for b in range(B):
        m = dict(com)
        m['xin'] = np.ascontiguousarray(x[:, b].transpose(1, 0, 2).reshape(C, T * N))
        in_maps.append(m)
    res = run_bass_kernel_spmd(_get_nc(), in_maps, list(range(B)),
                               trace=trace, tmpdir=tmpdir)
    out = np.empty((T, B, C, N), np.float32)
    for b in range(B):
        out[:, b] = res.results[b]['out'].reshape(T, C, N)
    return out.reshape(T * B, C, 16, 16), res.exec_time_ns


def kernel(**inputs):
    out, _ = run(inputs)
    return out


# revision 12
# speedup vs baseline: 1.5387x; 1.0908x over previous
"""Trainium2 Bass kernel for nn_Block_31954556682442 (spiking MoE-SSA block).

Sharding: pure data-parallel over batch B=8 -> one sample (4 LIF time steps)
per NeuronCore, zero collectives. v3 design:
  - ALL weight matmuls single-term bf16 (W and x both bf16-rounded); CPU-sim
    rel err 4.4e-3 vs the 2e-2 gate
  - kv-first attention: res_e = q_e^T (k v^T); kv integers <=256 exact bf16;
    k^T computed in v's layout by widening the v matmul rhs (no transposes)
  - attention-path LIF state kept in bf16 (values quantized to 0.5 and
    compare-safe), enabling 2x/4x DVE modes
  - depthwise 3x3 conv moved to the PE: per-(ch,tap) diagonal bf16 weights
    x zero-padded spike tiles; bias via diag @ ones
  - fc2 split: t-pair 0 accumulated across the ch loop (3 PSUM banks),
    t-pair 1 as a tail overlapped with the final LIF
  - LIF scans in 2^t-scaled form as in v2
Self-contained: hardcodes all shapes; no sibling imports.
"""
import numpy as np
import ml_dtypes

import concourse.bacc as bacc
import concourse.mybir as mybir
import concourse.tile as tile
from concourse.bass_utils import run_bass_kernel_spmd

F32 = mybir.dt.float32
BF16 = mybir.dt.bfloat16
AL = mybir.AluOpType
AF = mybir.ActivationFunctionType

T, B, C, N = 4, 8, 384, 256
ED = 96
NE = 4
HID, HH = 2048, 1024
S = float(1.0 / np.sqrt(1.0 + 1e-5))
P = 128


def _body(nc, tc, d):
    from contextlib import ExitStack
    VE = nc.vector
    GE = nc.gpsimd

    with ExitStack() as ctx:
        def pool(name, bufs, space="SBUF"):
            return ctx.enter_context(tc.tile_pool(name=name, bufs=bufs, space=space))

        wp = pool("wp", 1)
        mp = pool("mp", 1)
        xs_p = pool("xs_p", 3)       # (128,1024) f32 x, doubles as x_new
        xq_p = pool("xq_p", 3)       # (128,1024) bf16 rounded x
        xqt_p = pool("xqt_p", 3)     # (96,1024) f32 q pre-act
        xvt_p = pool("xvt_p", 3)     # (128,960) f32 v|k pre-act
        xrt_p = pool("xrt_p", 2)     # (128,8)
        spq_p = pool("spq_p", 4)     # (96,1024) bf16 q spikes
        vks_p = pool("vks_p", 4)     # (128,960) bf16 v|k spikes
        wsp_p = pool("wsp_p", 4)     # (128,8) f32 router spikes
        kv_p = pool("kv_p", 2)       # (96,384) bf16
        xr_p = pool("xr_p", 3)       # (128,768) bf16 res pre-act
        rs_p = pool("rs_p", 2)       # (128,768) bf16 masked res spikes
        y_p = pool("y_p", 4)         # (128,768) bf16
        ydn_p = pool("ydn_p", 3)     # (128,1024) bf16
        xp_p = pool("xp_p", 2)       # (128,768) f32 proj pre-act
        xh_p = pool("xh_p", 2)       # (128,2048) f32
        sp2_p = pool("sp2_p", 2)     # (128,1024) bf16 gate spikes
        xp1_p = pool("xp1_p", 1)     # padded dw spikes, 2 tiles
        acc_p = pool("acc_p", 2)     # (128,1024) f32 dw-conv acc
        mg0_p = pool("mg0_p", 2)     # (128,512) bf16 t-pair0 gated spikes
        mg1_p = pool("mg1_p", 8)     # (128,512) bf16 t-pair1 gated spikes
        mh_p = pool("mh_p", 2)       # (128,512) f32
        mdw_p = pool("mdw_p", 1)     # (128,256) f32
        xo_p = pool("xo_p", 2)       # (128,768) f32

        psum_ad = ExitStack()
        ps_ad = psum_ad.enter_context(
            tc.tile_pool(name="ps_ad", bufs=6, space="PSUM"))

        # ---------------- weight loads ----------------
        def wload(name, shape, dt=F32, src=None):
            w = wp.tile(shape, dt, name=name, tag=name)
            nc.sync.dma_start(out=w, in_=d[name] if src is None else src)
            return w

        ident = wload('ident', [P, P], BF16)
        # PE warmup: dummy matmuls to flip HAM to K=8/8 before phase A
        pwarm = ps_ad.tile([P, P], F32, name="pwarm", tag="pm")
        for wi in range(40):
            nc.tensor.matmul(pwarm, ident, ident, start=True, stop=True)
        warm_sink = wp.tile([P, 1], F32, name="warm_sink", tag="warm_sink")
        nc.scalar.activation(warm_sink, pwarm[:, 0:1], AF.Copy)

        # xs first (phase A starts on these)
        xs_kt = []
        for kt in range(3):
            x_ = xs_p.tile([P, 4 * N], F32, name=f"xs{kt}", tag="t")
            xs_kt.append(x_)
        for kt in range(3):
            nc.sync.dma_start(out=xs_kt[kt], in_=d['xin'][kt*P:(kt+1)*P, :])
        q_w, vk_w, pj_w, f1_w, f2_w, r_w = [], [], [], [], [], []
        for kt in range(3):
            q_w.append(wload(f'q_w{kt}', [P, 384], BF16, d['q_w'][kt*P:(kt+1)*P, :]))
            vk_w.append(wload(f'vk_w{kt}', [P, 480], BF16, d['vk_w'][kt*P:(kt+1)*P, :]))
            r_w.append(wload(f'r_w{kt}', [P, 4], F32, d['r_wT'][kt*P:(kt+1)*P, :]))
        sq = wload('s_q', [ED, 32])
        a_q, b_q = sq[:, 0:16], sq[:, 16:32]
        rb = wload('r_b', [1, 4]); ones = wload('ones', [1, P])
        for kt in range(3):
            pj_w.append(wload(f'pj_w{kt}', [P, 384], BF16, d['pj_w'][kt*P:(kt+1)*P, :]))
        spo = wload('s_po', [P, 48])
        a_p, b_p = spo[:, 0:12], spo[:, 12:24]
        a_o, b_o = spo[:, 24:36], spo[:, 36:48]
        for kt in range(3):
            f1_w.append(wload(f'f1_w{kt}', [P, 2048], BF16, d['f1_w'][kt*P:(kt+1)*P, :]))
        sh = wload('s_h', [P, 128])
        a_h, b_h = sh[:, 0:64], sh[:, 64:128]
        dwd = wload('dwd', [P, 80 * P], BF16)
        ones_b = wload('ones_b', [P, 512], BF16)
        for ch in range(8):
            f2_w.append(wload(f'f2_w{ch}', [P, 384], BF16, d['f2_w'][ch*P:(ch+1)*P, :]))

        # ---------------- x -> bf16 ----------------
        xq_kt = []
        for kt in range(3):
            q_ = xq_p.tile([P, 4 * N], BF16, name=f"xq{kt}", tag="t")
            nc.scalar.activation(q_, xs_kt[kt], AF.Copy)
            xq_kt.append(q_)

        # ---------------- phase A: q / v|k / router matmuls + evicts ----------------
        m_kq = mp.tile([ED, 4 * N], F32, name="m_kq", tag="m_kq")
        m_vt = mp.tile([P, 960], F32, name="m_vt", tag="m_vt")
        m_rt = mp.tile([P, 8], F32, name="m_rt", tag="m_rt")
        m_p = mp.tile([P, 768], F32, name="m_p", tag="m_p")
        m_o = [mp.tile([P, N], F32, name=f"m_o{i}", tag=f"m_o{i}") for i in range(3)]

        xq_t = [xqt_p.tile([ED, 4 * N], F32, name=f"xqt{t}", tag="t") for t in range(T)]
        xvt_t = [xvt_p.tile([P, 960], F32, name=f"xvt{t}", tag="t") for t in range(T)]
        xrt_t = [xrt_p.tile([P, 8], F32, name=f"xrt{t}", tag="t") for t in range(T)]

        for tp in range(2):
            for u in range(NE):
                pt = ps_ad.tile([ED, 512], F32, name=f"pq{u}_{tp}", tag="pm")
                for kt in range(3):
                    nc.tensor.matmul(pt, q_w[kt][:, 96*u:96*(u+1)],
                                     xq_kt[kt][:, tp*512:(tp+1)*512],
                                     start=(kt == 0), stop=(kt == 2))
                for ti in range(2):
                    t = tp * 2 + ti
                    c = u * 4 + t
                    nc.scalar.activation(xq_t[t][:, u*N:(u+1)*N], pt[:, ti*N:(ti+1)*N],
                                         AF.Identity, bias=b_q[:, c:c+1], scale=a_q[:, c:c+1])
        for t in range(T):
            for mt in range(2):
                pv = ps_ad.tile([P, 480], F32, name=f"pvt{t}_{mt}", tag="pm")
                for kt in range(3):
                    nc.tensor.matmul(pv, xq_kt[kt][:, t*N + mt*P: t*N + (mt+1)*P],
                                     vk_w[kt], start=(kt == 0), stop=(kt == 2))
                nc.scalar.activation(xvt_t[t][:, mt*480:(mt+1)*480], pv, AF.Copy,
                                     bias=0.0, scale=0.5 * float(2.0 ** t))
            for mt in range(2):
                pr = ps_ad.tile([P, 4], F32, name=f"prt{t}_{mt}", tag="pm")
                for kt in range(3):
                    nc.tensor.matmul(pr, xs_kt[kt][:, t*N + mt*P: t*N + (mt+1)*P],
                                     r_w[kt], start=(kt == 0), stop=False)
                nc.tensor.matmul(pr, ones, rb, start=False, stop=True)
                nc.scalar.activation(xrt_t[t][:, mt*4:(mt+1)*4], pr, AF.Copy,
                                     bias=0.0, scale=float(2.0 ** t))

        # ---------------- phase B: LIF scans for q / v|k / r ----------------
        sp_q, vk_sp, w_sp = [], [], []
        for t in range(T):
            thr = float(2.0 ** t)
            U = xq_t[t]
            if t > 0:
                GE.tensor_add(U, m_kq, U)
            sp = spq_p.tile([ED, 4 * N], BF16, name=f"spq{t}", tag="t")
            VE.tensor_single_scalar(sp, U, thr, AL.is_ge)
            if t < T - 1:
                VE.scalar_tensor_tensor(out=m_kq, in0=U, scalar=thr, in1=U,
                                        op0=AL.is_lt, op1=AL.mult)
            sp_q.append(sp)

            U = xvt_t[t]
            if t > 0:
                GE.tensor_add(U, m_vt, U)
            vs = vks_p.tile([P, 960], BF16, name=f"vks{t}", tag="t")
            VE.tensor_single_scalar(vs, U, thr, AL.is_ge)
            if t < T - 1:
                VE.scalar_tensor_tensor(out=m_vt, in0=U, scalar=thr, in1=U,
                                        op0=AL.is_lt, op1=AL.mult)
            vk_sp.append(vs)

            U = xrt_t[t]
            if t > 0:
                GE.tensor_add(U, m_rt, U)
            ws = wsp_p.tile([P, 8], F32, name=f"wsp{t}", tag="t")
            VE.tensor_single_scalar(ws, U, thr, AL.is_ge)
            if t < T - 1:
                VE.scalar_tensor_tensor(out=m_rt, in0=U, scalar=thr, in1=U,
                                        op0=AL.is_lt, op1=AL.mult)
            w_sp.append(ws)

        # ---------------- phase C: kv-first experts ----------------
        # masked spikes: w_e*(U>=thr) via one tensor_scalar (4x mode);
        # y accumulated with plain bf16 tensor_tensor adds (2x mode)
        y = [None] * T
        m_res_e = [mp.tile([P, 768], BF16, name=f"m_res{e}", tag=f"m_res{e}")
                   for e in range(NE)]
        for t in range(T):
            thr = float(2.0 ** t)
            pkv = ps_ad.tile([ED, 384], F32, name=f"pkv{t}", tag="pm")
            for mt in range(2):
                nc.tensor.matmul(pkv, vk_sp[t][:, mt*480+384:(mt+1)*480],
                                 vk_sp[t][:, mt*480:mt*480+384],
                                 start=(mt == 0), stop=(mt == 1))
            kv = kv_p.tile([ED, 384], BF16, name=f"kv{t}", tag="t")
            nc.scalar.activation(kv, pkv, AF.Copy)
            yt = y_p.tile([P, 768], BF16, name=f"y{t}", tag="t")
            y[t] = yt
            for e in range(NE):
                m_res = m_res_e[e]
                xr = xr_p.tile([P, 768], BF16, name=f"xres{e}{t}", tag="t")
                for mt in range(2):
                    pr_ = ps_ad.tile([P, 384], F32, name=f"pres{e}{t}{mt}", tag="pm")
                    nc.tensor.matmul(pr_, sp_q[t][:, e*N + mt*P: e*N + (mt+1)*P],
                                     kv, start=True, stop=True)
                    nc.scalar.activation(xr[:, mt*384:(mt+1)*384], pr_, AF.Copy,
                                         bias=0.0, scale=0.5 * thr)
                U = xr
                if t > 0:
                    GE.tensor_add(U, m_res, U)
                dst = yt if e == 0 else rs_p.tile([P, 768], BF16,
                                                  name=f"rsm{e}{t}", tag="t")
                for mt in range(2):
                    VE.tensor_scalar(out=dst[:, mt*384:(mt+1)*384],
                                     in0=U[:, mt*384:(mt+1)*384],
                                     scalar1=thr,
                                     scalar2=w_sp[t][:, mt*4+e:mt*4+e+1],
                                     op0=AL.is_ge, op1=AL.mult)
                if t < T - 1:
                    VE.scalar_tensor_tensor(out=m_res, in0=U, scalar=thr, in1=U,
                                            op0=AL.is_lt, op1=AL.mult)
                if e > 0:
                    VE.tensor_add(yt, yt, dst)

        # ---------------- phase D: transpose y, proj, LIF, residual ----------------
        ydn = [ydn_p.tile([P, 4 * N], BF16, name=f"ydn{dt}", tag="t") for dt in range(3)]
        xp_t = [xp_p.tile([P, 768], F32, name=f"xp{t}", tag="t") for t in range(T)]
        for tp in range(2):
            for t in (tp * 2, tp * 2 + 1):
                for mt in range(2):
                    for dt in range(3):
                        ptr = ps_ad.tile([P, P], BF16, name=f"ptr{t}{mt}{dt}", tag="pm")
                        nc.tensor.transpose(
                            ptr, y[t][:, mt*384 + dt*P: mt*384 + (dt+1)*P], ident)
                        nc.scalar.activation(ydn[dt][:, t*N + mt*P: t*N + (mt+1)*P],
                                             ptr, AF.Copy)
            for mt in range(3):
                pp = ps_ad.tile([P, 512], F32, name=f"pp{mt}_{tp}", tag="pm")
                for kt in range(3):
                    nc.tensor.matmul(pp, pj_w[kt][:, mt*P:(mt+1)*P],
                                     ydn[kt][:, tp*512:(tp+1)*512],
                                     start=(kt == 0), stop=(kt == 2))
                for ti in range(2):
                    t = tp * 2 + ti
                    c = mt * 4 + t
                    nc.scalar.activation(xp_t[t][:, mt*N:(mt+1)*N], pp[:, ti*N:(ti+1)*N],
                                         AF.Identity, bias=b_p[:, c:c+1], scale=a_p[:, c:c+1])
            for t in (tp * 2, tp * 2 + 1):
                thr = float(2.0 ** t)
                U = xp_t[t]
                if t > 0:
                    GE.tensor_add(U, m_p, U)
                if t < T - 1:
                    VE.scalar_tensor_tensor(out=m_p, in0=U, scalar=thr, in1=U,
                                            op0=AL.is_lt, op1=AL.mult)
                for mt in range(3):
                    # x_new overwrites xs in place (residual add)
                    VE.scalar_tensor_tensor(
                        out=xs_kt[mt][:, t*N:(t+1)*N], in0=U[:, mt*N:(mt+1)*N],
                        scalar=thr, in1=xs_kt[mt][:, t*N:(t+1)*N],
                        op0=AL.is_ge, op1=AL.add)
            # x_new -> bf16 for this time-pair (fc1 can start on tp=0)
            for kt in range(3):
                nc.scalar.activation(xq_kt[kt][:, tp*512:(tp+1)*512],
                                     xs_kt[kt][:, tp*512:(tp+1)*512], AF.Copy)

        # ---------------- phase E: MLP ----------------
        psum_ad.close()
        ps_e = ctx.enter_context(tc.tile_pool(name="ps_e", bufs=3, space="PSUM"))
        ps_dw = ctx.enter_context(tc.tile_pool(name="ps_dw", bufs=2, space="PSUM"))
        ps_po = ctx.enter_context(tc.tile_pool(name="ps_po", bufs=3, space="PSUM"))
        # padded dw-spike tiles (zero borders written once)
        xp1_bufs = [xp1_p.tile([P, 1296], BF16, name=f"xp1_{i}", tag=f"xp1_{i}")
                    for i in range(2)]
        for b_ in xp1_bufs:
            GE.memset(b_, 0.0)
        po1 = [ps_po.tile([P, 512], F32, name=f"po1_{mt}", tag="po")
               for mt in range(3)]
        TAPS = [(dy, dx) for dy in range(3) for dx in range(3)]
        mg_t = []
        for ch in range(8):
            xh = xh_p.tile([P, 2048], F32, name=f"xh{ch}", tag="t")
            for half in range(2):
                mth = ch + 8 * half
                for tp in range(2):
                    ph = ps_e.tile([P, 512], F32, name=f"ph{ch}{half}{tp}", tag="pm")
                    for kt in range(3):
                        nc.tensor.matmul(ph, f1_w[kt][:, mth*P:(mth+1)*P],
                                         xq_kt[kt][:, tp*512:(tp+1)*512],
                                         start=(kt == 0), stop=(kt == 2))
                    for ti in range(2):
                        t = tp * 2 + ti
                        c = mth * 4 + t
                        nc.scalar.activation(
                            xh[:, half*1024 + t*N: half*1024 + (t+1)*N],
                            ph[:, ti*N:(ti+1)*N], AF.Identity,
                            bias=b_h[:, c:c+1], scale=a_h[:, c:c+1])
            # h-LIF over t; dw-half spikes written into padded tile, gate into sp2
            m_h = mh_p.tile([P, 512], F32, name=f"m_h{ch}", tag="t")
            sp2 = sp2_p.tile([P, 1024], BF16, name=f"sp2_{ch}", tag="t")
            xp1 = xp1_bufs[ch % 2]
            xh3 = xh.rearrange("p (h q) -> p h q", h=2)
            mh3 = m_h.rearrange("p (h q) -> p h q", h=2)
            xh4 = xh.rearrange("p (a h w) -> p a h w", h=16, w=16)
            xp4 = xp1.rearrange("p (t h w) -> p t h w", t=4, h=18, w=18)
            for t in range(T):
                thr = float(2.0 ** t)
                U3 = xh3[:, :, t*N:(t+1)*N]
                if t > 0:
                    GE.tensor_add(U3, mh3, U3)
                VE.tensor_single_scalar(xp4[:, t, 1:17, 1:17], xh4[:, t], thr, AL.is_ge)
                VE.tensor_single_scalar(sp2[:, t*N:(t+1)*N],
                                        xh[:, 1024 + t*N: 1024 + (t+1)*N], thr, AL.is_ge)
                if t < T - 1:
                    VE.scalar_tensor_tensor(out=mh3, in0=U3, scalar=thr, in1=U3,
                                            op0=AL.is_lt, op1=AL.mult)
            # depthwise conv on the PE: diag(bias) @ ones + sum diag(tap) @ shifted
            acc = acc_p.tile([P, 1024], F32, name=f"acc{ch}", tag="t")
            for tb in range(2):
                pa = ps_dw.tile([P, 512], F32, name=f"dwa{ch}{tb}", tag="pdw")
                nc.tensor.matmul(pa, dwd[:, (ch*10+9)*P:(ch*10+10)*P], ones_b,
                                 start=True, stop=False)
                for j, (dy, dx) in enumerate(TAPS):
                    rhs = xp4[:, tb*2:(tb+1)*2, dy:dy+16, dx:dx+16]
                    nc.tensor.matmul(pa, dwd[:, (ch*10+j)*P:(ch*10+j+1)*P], rhs,
                                     start=False, stop=(j == 8))
                nc.scalar.activation(acc[:, tb*512:(tb+1)*512], pa, AF.Copy)
            # dw-LIF + gate -> mg0 (t-pair 0) / mg1 (t-pair 1), bf16
            m_dw = mdw_p.tile([P, N], F32, name=f"m_dw{ch}", tag="t")
            mg0 = mg0_p.tile([P, 512], BF16, name=f"mg0_{ch}", tag="t")
            mg1 = mg1_p.tile([P, 512], BF16, name=f"mg1_{ch}", tag="t")
            for t in range(T):
                thr = float(2.0 ** t)
                mg = mg0 if t < 2 else mg1
                U = acc[:, t*N:(t+1)*N]
                if t > 0:
                    VE.scalar_tensor_tensor(out=U, in0=U, scalar=thr, in1=m_dw,
                                            op0=AL.mult, op1=AL.add)
                VE.scalar_tensor_tensor(out=mg[:, (t % 2)*N:(t % 2 + 1)*N], in0=U,
                                        scalar=thr, in1=sp2[:, t*N:(t+1)*N],
                                        op0=AL.is_ge, op1=AL.mult)
                if t < T - 1:
                    VE.scalar_tensor_tensor(out=m_dw, in0=U, scalar=thr, in1=U,
                                            op0=AL.is_lt, op1=AL.mult)
            mg_t.append(mg1)
            # fc2 t-pair 0 accumulate
            for mt in range(3):
                nc.tensor.matmul(po1[mt], f2_w[ch][:, mt*P:(mt+1)*P],
                                 mg0,
                                 start=(ch == 0), stop=(ch == 7), skip_group_check=True)

        # fc2 t-pair 1 tail
        po2 = [ps_po.tile([P, 512], F32, name=f"po2_{mt}", tag="po")
               for mt in range(3)]
        for mt in range(3):
            for ch in range(8):
                nc.tensor.matmul(po2[mt], f2_w[ch][:, mt*P:(mt+1)*P],
                                 mg_t[ch],
                                 start=(ch == 0), stop=(ch == 7), skip_group_check=True)

        # fc2 evict + final LIF + residual + store
        xo_t = [xo_p.tile([P, 768], F32, name=f"xo{t}", tag="t") for t in range(T)]
        for t in range(T):
            po = po1 if t < 2 else po2
            for mt in range(3):
                c = mt * 4 + t
                nc.scalar.activation(xo_t[t][:, mt*N:(mt+1)*N],
                                     po[mt][:, (t % 2)*N:(t % 2 + 1)*N],
                                     AF.Identity, bias=b_o[:, c:c+1], scale=a_o[:, c:c+1])
        for t in range(T):
            thr = float(2.0 ** t)
            for mt in range(3):
                U = xo_t[t][:, mt*N:(mt+1)*N]
                if t > 0:
                    GE.tensor_add(U, m_o[mt], U)
                if t < T - 1:
                    VE.scalar_tensor_tensor(out=m_o[mt], in0=U, scalar=thr, in1=U,
                                            op0=AL.is_lt, op1=AL.mult)
                # final out in place over xo (reset already consumed U)
                VE.scalar_tensor_tensor(
                    out=U, in0=U, scalar=thr,
                    in1=xs_kt[mt][:, t*N:(t+1)*N], op0=AL.is_ge, op1=AL.add)
                nc.sync.dma_start(out=d['out'][t*C + mt*P: t*C + (mt+1)*P, :],
                                  in_=U)


def _build():
    nc = bacc.Bacc()
    with tile.TileContext(nc) as tc:
        with tc.tile_pool(name="dram", bufs=1, space="DRAM") as dram:
            def din(name, shape, dt=F32):
                return dram.tile(shape, dt, kind="ExternalInput", name=name,
                                 uniquify=False)
            d = {
                'xin': din('xin', [C, 4 * N]),
                'out': dram.tile([T * C, N], F32, kind="ExternalOutput",
                                 name='out', uniquify=False),
                'q_w': din('q_w', [384, 384], BF16),
                's_q': din('s_q', [ED, 32]),
                'vk_w': din('vk_w', [384, 480], BF16),
                'r_wT': din('r_wT', [384, 4]),
                'r_b': din('r_b', [1, 4]),
                'ones': din('ones', [1, 128]),
                'pj_w': din('pj_w', [384, 384], BF16),
                's_po': din('s_po', [128, 48]),
                'f1_w': din('f1_w', [384, 2048], BF16),
                's_h': din('s_h', [128, 128]),
                'dwd': din('dwd', [128, 80 * 128], BF16),
                'ones_b': din('ones_b', [128, 512], BF16),
                'f2_w': din('f2_w', [1024, 384], BF16),
                'ident': din('ident', [128, 128], BF16),
            }
            _body(nc, tc, d)
    nc.finalize()
    return nc


_NC_CACHE = {}


def _get_nc():
    if 'nc' not in _NC_CACHE:
        _NC_CACHE['nc'] = _build()
    return _NC_CACHE['nc']


def _tcols(a):
    rows, k = a.shape
    out = np.empty((rows, k * 4), np.float32)
    for u in range(k):
        for t in range(4):
            out[:, u * 4 + t] = a[:, u] * (2.0 ** t)
    return out


def _prep_common(inputs):
    inp = {k: np.asarray(v, np.float32) for k, v in inputs.items()}
    bf16 = ml_dtypes.bfloat16

    exp_wT = np.concatenate([inp['exp_w'][e].T for e in range(NE)], axis=1)
    a_q = np.zeros((ED, NE), np.float32)
    b_q = np.zeros((ED, NE), np.float32)
    for e in range(NE):
        a_q[:, e] = 0.5 * inp['exp_g'][e] * S
        b_q[:, e] = 0.5 * inp['exp_b'][e]
    vk = np.concatenate([inp['v_w'].T, inp['k_w'].T], axis=1)

    g = inp['dw_g']
    taps = (inp['dw_w'][:, 0] * (0.5 * g * S)[:, None, None]).reshape(HH, 9)
    bias = 0.5 * (inp['dw_b'] * g * S + inp['dw_be'])
    dwd = np.zeros((P, 80, P), np.float32)
    pi = np.arange(P)
    for ch in range(8):
        cg = ch * P + pi
        for j in range(9):
            dwd[pi, ch*10+j, pi] = taps[cg, j]
        dwd[pi, ch*10+9, pi] = bias[cg]

    com = {
        'q_w': exp_wT.astype(bf16),
        's_q': np.concatenate([_tcols(a_q), _tcols(b_q)], axis=1),
        'vk_w': vk.astype(bf16),
        'r_wT': inp['router_w'].T * (inp['router_g'] * S * 0.5)[None, :],
        'r_b': (0.5 * (inp['router_b'] * inp['router_g'] * S
                       + inp['router_be'])).reshape(1, 4),
        'ones': np.ones((1, 128), np.float32),
        'pj_w': inp['proj_w'].T.astype(bf16),
        's_po': np.concatenate([
            _tcols((0.5 * inp['proj_g'] * S).reshape(3, 128).T),
            _tcols((0.5 * (inp['proj_b'] * inp['proj_g'] * S
                           + inp['proj_be'])).reshape(3, 128).T),
            _tcols((0.5 * inp['fc2_g'] * S).reshape(3, 128).T),
            _tcols((0.5 * (inp['fc2_b'] * inp['fc2_g'] * S
                           + inp['fc2_be'])).reshape(3, 128).T)], axis=1),
        'f1_w': inp['fc1_w'].T.astype(bf16),
        's_h': np.concatenate([
            _tcols((0.5 * inp['fc1_g'] * S).reshape(16, 128).T),
            _tcols((0.5 * (inp['fc1_b'] * inp['fc1_g'] * S
                           + inp['fc1_be'])).reshape(16, 128).T)], axis=1),
        'dwd': dwd.reshape(P, 80 * P).astype(bf16),
        'ones_b': np.ones((P, 512), bf16),
        'f2_w': inp['fc2_w'].T.astype(bf16),
        'ident': np.eye(128, dtype=bf16),
    }
    return {k: np.ascontiguousarray(v) for k, v in com.items()}


def run(inputs, trace=False, tmpdir=None):
    com = _prep_common(inputs)
    x = np.asarray(inputs['x'], np.float32).reshape(T, B, C, N)
    in_maps = []
    for b in range(B):
        m = dict(com)
        m['xin'] = np.ascontiguousarray(x[:, b].transpose(1, 0, 2).reshape(C, T * N))
        in_maps.append(m)
    res = run_bass_kernel_spmd(_get_nc(), in_maps, list(range(B)),
                               trace=trace, tmpdir=tmpdir)
    out = np.empty((T, B, C, N), np.float32)
    for b in range(B):
        out[:, b] = res.results[b]['out'].reshape(T, C, N)
    return out.reshape(T * B, C, 16, 16), res.exec_time_ns


def kernel(**inputs):
    out, _ = run(inputs)
    return out


# revision 16
# speedup vs baseline: 1.5741x; 1.0230x over previous
"""Trainium2 Bass kernel for nn_Block_31954556682442 (spiking MoE-SSA block).

Sharding: pure data-parallel over batch B=8 -> one sample (4 LIF time steps)
per NeuronCore, zero collectives. v3 design:
  - ALL weight matmuls single-term bf16 (W and x both bf16-rounded); CPU-sim
    rel err 4.4e-3 vs the 2e-2 gate
  - kv-first attention: res_e = q_e^T (k v^T); kv integers <=256 exact bf16;
    k^T computed in v's layout by widening the v matmul rhs (no transposes)
  - attention-path LIF state kept in bf16 (values quantized to 0.5 and
    compare-safe), enabling 2x/4x DVE modes
  - depthwise 3x3 conv moved to the PE: per-(ch,tap) diagonal bf16 weights
    x zero-padded spike tiles; bias via diag @ ones
  - fc2 split: t-pair 0 accumulated across the ch loop (3 PSUM banks),
    t-pair 1 as a tail overlapped with the final LIF
  - LIF scans in 2^t-scaled form as in v2
Self-contained: hardcodes all shapes; no sibling imports.
"""
import numpy as np
import ml_dtypes

import concourse.bacc as bacc
import concourse.mybir as mybir
import concourse.tile as tile
from concourse.bass_utils import run_bass_kernel_spmd

F32 = mybir.dt.float32
BF16 = mybir.dt.bfloat16
AL = mybir.AluOpType
AF = mybir.ActivationFunctionType

T, B, C, N = 4, 8, 384, 256
ED = 96
NE = 4
HID, HH = 2048, 1024
S = float(1.0 / np.sqrt(1.0 + 1e-5))
P = 128


def _body(nc, tc, d):
    from contextlib import ExitStack
    VE = nc.vector
    GE = nc.gpsimd

    with ExitStack() as ctx:
        def pool(name, bufs, space="SBUF"):
            return ctx.enter_context(tc.tile_pool(name=name, bufs=bufs, space=space))

        wp = pool("wp", 1)
        mp = pool("mp", 1)
        xs_p = pool("xs_p", 3)       # (128,1024) f32 x, doubles as x_new
        xq_p = pool("xq_p", 3)       # (128,1024) bf16 rounded x
        xqt_p = pool("xqt_p", 3)     # (96,1024) f32 q pre-act
        xvt_p = pool("xvt_p", 3)     # (128,960) f32 v|k pre-act
        xrt_p = pool("xrt_p", 2)     # (128,8)
        spq_p = pool("spq_p", 4)     # (96,1024) bf16 q spikes
        vks_p = pool("vks_p", 4)     # (128,960) bf16 v|k spikes
        wsp_p = pool("wsp_p", 4)     # (128,8) f32 router spikes
        kv_p = pool("kv_p", 2)       # (96,384) bf16
        xr_p = pool("xr_p", 3)       # (128,768) bf16 res pre-act
        rs_p = pool("rs_p", 2)       # (128,768) bf16 masked res spikes
        y_p = pool("y_p", 4)         # (128,768) bf16
        ydn_p = pool("ydn_p", 3)     # (128,1024) bf16
        xp_p = pool("xp_p", 2)       # (128,768) f32 proj pre-act
        xh_p = pool("xh_p", 2)       # (128,2048) f32
        sp2_p = pool("sp2_p", 2)     # (128,1024) bf16 gate spikes
        xp1_p = pool("xp1_p", 1)     # padded dw spikes, 2 tiles
        acc_p = pool("acc_p", 2)     # (128,1024) f32 dw-conv acc
        mg0_p = pool("mg0_p", 2)     # (128,512) bf16 t-pair0 gated spikes
        mg1_p = pool("mg1_p", 8)     # (128,512) bf16 t-pair1 gated spikes
        mh_p = pool("mh_p", 2)       # (128,512) f32
        mdw_p = pool("mdw_p", 1)     # (128,256) f32
        xo_p = pool("xo_p", 2)       # (128,768) f32

        psum_ad = ExitStack()
        ps_ad = psum_ad.enter_context(
            tc.tile_pool(name="ps_ad", bufs=6, space="PSUM"))

        # ---------------- weight loads ----------------
        def wload(name, shape, dt=F32, src=None):
            w = wp.tile(shape, dt, name=name, tag=name)
            nc.sync.dma_start(out=w, in_=d[name] if src is None else src)
            return w

        ident = wload('ident', [P, P], BF16)
        # PE warmup: dummy matmuls to flip HAM to K=8/8 before phase A
        pwarm = ps_ad.tile([P, P], F32, name="pwarm", tag="pm")
        for wi in range(24):
            nc.tensor.matmul(pwarm, ident, ident, start=True, stop=True)
        warm_sink = wp.tile([P, 1], F32, name="warm_sink", tag="warm_sink")
        nc.scalar.activation(warm_sink, pwarm[:, 0:1], AF.Copy)

        # xs first (phase A starts on these)
        xs_kt = []
        for kt in range(3):
            x_ = xs_p.tile([P, 4 * N], F32, name=f"xs{kt}", tag="t")
            xs_kt.append(x_)
        for kt in range(3):
            nc.sync.dma_start(out=xs_kt[kt], in_=d['xin'][kt*P:(kt+1)*P, :])
        q_w, vk_w, pj_w, f1_w, f2_w = [], [], [], [], []
        for kt in range(3):
            q_w.append(wload(f'q_w{kt}', [P, 384], BF16, d['q_w'][kt*P:(kt+1)*P, :]))
            vk_w.append(wload(f'vk_w{kt}', [P, 484], BF16, d['vk_w'][kt*P:(kt+1)*P, :]))
        sq = wload('s_q', [ED, 32])
        a_q, b_q = sq[:, 0:16], sq[:, 16:32]
        rb2 = wload('rb2', [1, 484], BF16); ones = wload('ones', [1, P], BF16)
        for kt in range(3):
            pj_w.append(wload(f'pj_w{kt}', [P, 384], BF16, d['pj_w'][kt*P:(kt+1)*P, :]))
        spo = wload('s_po', [P, 48])
        a_p, b_p = spo[:, 0:12], spo[:, 12:24]
        a_o, b_o = spo[:, 24:36], spo[:, 36:48]
        for kt in range(3):
            f1_w.append(wload(f'f1_w{kt}', [P, 2048], BF16, d['f1_w'][kt*P:(kt+1)*P, :]))
        sh = wload('s_h', [P, 128])
        a_h, b_h = sh[:, 0:64], sh[:, 64:128]
        dwd = wload('dwd', [P, 72 * P], BF16)
        bdw = wload('b_dw', [P, 8])
        for ch in range(8):
            f2_w.append(wload(f'f2_w{ch}', [P, 384], BF16, d['f2_w'][ch*P:(ch+1)*P, :]))

        # ---------------- x -> bf16 ----------------
        xq_kt = []
        for kt in range(3):
            q_ = xq_p.tile([P, 4 * N], BF16, name=f"xq{kt}", tag="t")
            nc.scalar.activation(q_, xs_kt[kt], AF.Copy)
            xq_kt.append(q_)

        # ---------------- phase A: q / v|k / router matmuls + evicts ----------------
        m_kq = mp.tile([ED, 4 * N], F32, name="m_kq", tag="m_kq")
        m_vt = mp.tile([P, 968], F32, name="m_vt", tag="m_vt")
        m_p = mp.tile([P, 768], F32, name="m_p", tag="m_p")
        m_o = [mp.tile([P, N], F32, name=f"m_o{i}", tag=f"m_o{i}") for i in range(3)]

        xq_t = [xqt_p.tile([ED, 4 * N], F32, name=f"xqt{t}", tag="t") for t in range(T)]
        xvt_t = [xvt_p.tile([P, 968], F32, name=f"xvt{t}", tag="t") for t in range(T)]

        for tp in range(2):
            for u in range(NE):
                pt = ps_ad.tile([ED, 512], F32, name=f"pq{u}_{tp}", tag="pm")
                for kt in range(3):
                    nc.tensor.matmul(pt, q_w[kt][:, 96*u:96*(u+1)],
                                     xq_kt[kt][:, tp*512:(tp+1)*512],
                                     start=(kt == 0), stop=(kt == 2))
                for ti in range(2):
                    t = tp * 2 + ti
                    c = u * 4 + t
                    nc.scalar.activation(xq_t[t][:, u*N:(u+1)*N], pt[:, ti*N:(ti+1)*N],
                                         AF.Identity, bias=b_q[:, c:c+1], scale=a_q[:, c:c+1])
        for t in range(T):
            for mt in range(2):
                pv = ps_ad.tile([P, 484], F32, name=f"pvt{t}_{mt}", tag="pm")
                for kt in range(3):
                    nc.tensor.matmul(pv, xq_kt[kt][:, t*N + mt*P: t*N + (mt+1)*P],
                                     vk_w[kt], start=(kt == 0), stop=False,
                                     skip_group_check=True)
                nc.tensor.matmul(pv[:, 480:484], ones, rb2[:, 480:484],
                                 start=False, stop=True, skip_group_check=True)
                nc.scalar.activation(xvt_t[t][:, mt*484:(mt+1)*484], pv, AF.Copy,
                                     bias=0.0, scale=0.5 * float(2.0 ** t))

        # ---------------- phase B: LIF scans for q / v|k / r ----------------
        sp_q, vk_sp, w_sp = [], [], []
        for t in range(T):
            thr = float(2.0 ** t)
            U = xq_t[t]
            if t > 0:
                GE.tensor_add(U, m_kq, U)
            sp = spq_p.tile([ED, 4 * N], BF16, name=f"spq{t}", tag="t")
            VE.tensor_single_scalar(sp, U, thr, AL.is_ge)
            if t < T - 1:
                VE.scalar_tensor_tensor(out=m_kq, in0=U, scalar=thr, in1=U,
                                        op0=AL.is_lt, op1=AL.mult)
            sp_q.append(sp)

            U = xvt_t[t]
            if t > 0:
                GE.tensor_add(U, m_vt, U)
            vs = vks_p.tile([P, 968], BF16, name=f"vks{t}", tag="t")
            VE.tensor_single_scalar(vs, U, thr, AL.is_ge)
            # router spike columns as f32 (tensor_scalar scalar2 operand)
            ws = wsp_p.tile([P, 8], F32, name=f"wsp{t}", tag="t")
            U3 = U.rearrange("p (m c) -> p m c", m=2)
            VE.tensor_single_scalar(ws.rearrange("p (m c) -> p m c", m=2),
                                    U3[:, :, 480:484], thr, AL.is_ge)
            w_sp.append(ws)
            if t < T - 1:
                VE.scalar_tensor_tensor(out=m_vt, in0=U, scalar=thr, in1=U,
                                        op0=AL.is_lt, op1=AL.mult)
            vk_sp.append(vs)

        # ---------------- phase C: kv-first experts ----------------
        # masked spikes: w_e*(U>=thr) via one tensor_scalar (4x mode);
        # y accumulated with plain bf16 tensor_tensor adds (2x mode)
        y = [None] * T
        m_res_e = [mp.tile([P, 768], BF16, name=f"m_res{e}", tag=f"m_res{e}")
                   for e in range(NE)]
        for t in range(T):
            thr = float(2.0 ** t)
            pkv = ps_ad.tile([ED, 384], F32, name=f"pkv{t}", tag="pm")
            for mt in range(2):
                nc.tensor.matmul(pkv, vk_sp[t][:, mt*484+384:mt*484+480],
                                 vk_sp[t][:, mt*484:mt*484+384],
                                 start=(mt == 0), stop=(mt == 1))
            kv = kv_p.tile([ED, 384], BF16, name=f"kv{t}", tag="t")
            nc.scalar.activation(kv, pkv, AF.Copy)
            yt = y_p.tile([P, 768], BF16, name=f"y{t}", tag="t")
            y[t] = yt
            for e in range(NE):
                m_res = m_res_e[e]
                xr = xr_p.tile([P, 768], BF16, name=f"xres{e}{t}", tag="t")
                for mt in range(2):
                    pr_ = ps_ad.tile([P, 384], F32, name=f"pres{e}{t}{mt}", tag="pm")
                    nc.tensor.matmul(pr_, sp_q[t][:, e*N + mt*P: e*N + (mt+1)*P],
                                     kv, start=True, stop=True)
                    nc.scalar.activation(xr[:, mt*384:(mt+1)*384], pr_, AF.Copy,
                                         bias=0.0, scale=0.5 * thr)
                U = xr
                if t > 0:
                    GE.tensor_add(U, m_res, U)
                dst = yt if e == 0 else rs_p.tile([P, 768], BF16,
                                                  name=f"rsm{e}{t}", tag="t")
                for mt in range(2):
                    VE.tensor_scalar(out=dst[:, mt*384:(mt+1)*384],
                                     in0=U[:, mt*384:(mt+1)*384],
                                     scalar1=thr,
                                     scalar2=w_sp[t][:, mt*4+e:mt*4+e+1],
                                     op0=AL.is_ge, op1=AL.mult)
                if t < T - 1:
                    VE.scalar_tensor_tensor(out=m_res, in0=U, scalar=thr, in1=U,
                                            op0=AL.is_lt, op1=AL.mult)
                if e > 0:
                    VE.tensor_add(yt, yt, dst)

        # ---------------- phase D: transpose y, proj, LIF, residual ----------------
        ydn = [ydn_p.tile([P, 4 * N], BF16, name=f"ydn{dt}", tag="t") for dt in range(3)]
        xp_t = [xp_p.tile([P, 768], F32, name=f"xp{t}", tag="t") for t in range(T)]
        for tp in range(2):
            for t in (tp * 2, tp * 2 + 1):
                for mt in range(2):
                    for dt in range(3):
                        ptr = ps_ad.tile([P, P], BF16, name=f"ptr{t}{mt}{dt}", tag="pm")
                        nc.tensor.transpose(
                            ptr, y[t][:, mt*384 + dt*P: mt*384 + (dt+1)*P], ident)
                        nc.scalar.activation(ydn[dt][:, t*N + mt*P: t*N + (mt+1)*P],
                                             ptr, AF.Copy)
            for mt in range(3):
                pp = ps_ad.tile([P, 512], F32, name=f"pp{mt}_{tp}", tag="pm")
                for kt in range(3):
                    nc.tensor.matmul(pp, pj_w[kt][:, mt*P:(mt+1)*P],
                                     ydn[kt][:, tp*512:(tp+1)*512],
                                     start=(kt == 0), stop=(kt == 2))
                for ti in range(2):
                    t = tp * 2 + ti
                    c = mt * 4 + t
                    nc.scalar.activation(xp_t[t][:, mt*N:(mt+1)*N], pp[:, ti*N:(ti+1)*N],
                                         AF.Identity, bias=b_p[:, c:c+1], scale=a_p[:, c:c+1])
            for t in (tp * 2, tp * 2 + 1):
                thr = float(2.0 ** t)
                U = xp_t[t]
                if t > 0:
                    GE.tensor_add(U, m_p, U)
                if t < T - 1:
                    VE.scalar_tensor_tensor(out=m_p, in0=U, scalar=thr, in1=U,
                                            op0=AL.is_lt, op1=AL.mult)
                for mt in range(3):
                    # x_new overwrites xs in place (residual add)
                    VE.scalar_tensor_tensor(
                        out=xs_kt[mt][:, t*N:(t+1)*N], in0=U[:, mt*N:(mt+1)*N],
                        scalar=thr, in1=xs_kt[mt][:, t*N:(t+1)*N],
                        op0=AL.is_ge, op1=AL.add)
            # x_new -> bf16 for this time-pair (fc1 can start on tp=0)
            for kt in range(3):
                nc.scalar.activation(xq_kt[kt][:, tp*512:(tp+1)*512],
                                     xs_kt[kt][:, tp*512:(tp+1)*512], AF.Copy)

        # ---------------- phase E: MLP ----------------
        psum_ad.close()
        ps_e = ctx.enter_context(tc.tile_pool(name="ps_e", bufs=3, space="PSUM"))
        ps_dw = ctx.enter_context(tc.tile_pool(name="ps_dw", bufs=2, space="PSUM"))
        ps_po = ctx.enter_context(tc.tile_pool(name="ps_po", bufs=3, space="PSUM"))
        # padded dw-spike tiles (zero borders written once)
        xp1_bufs = [xp1_p.tile([P, 1296], BF16, name=f"xp1_{i}", tag=f"xp1_{i}")
                    for i in range(2)]
        for b_ in xp1_bufs:
            GE.memset(b_, 0.0)
        po1 = [ps_po.tile([P, 512], F32, name=f"po1_{mt}", tag="po")
               for mt in range(3)]
        TAPS = [(dy, dx) for dy in range(3) for dx in range(3)]
        mg_t = []
        for ch in range(8):
            xh = xh_p.tile([P, 2048], F32, name=f"xh{ch}", tag="t")
            for half in range(2):
                mth = ch + 8 * half
                for tp in range(2):
                    ph = ps_e.tile([P, 512], F32, name=f"ph{ch}{half}{tp}", tag="pm")
                    for kt in range(3):
                        nc.tensor.matmul(ph, f1_w[kt][:, mth*P:(mth+1)*P],
                                         xq_kt[kt][:, tp*512:(tp+1)*512],
                                         start=(kt == 0), stop=(kt == 2))
                    for ti in range(2):
                        t = tp * 2 + ti
                        c = mth * 4 + t
                        nc.scalar.activation(
                            xh[:, half*1024 + t*N: half*1024 + (t+1)*N],
                            ph[:, ti*N:(ti+1)*N], AF.Identity,
                            bias=b_h[:, c:c+1], scale=a_h[:, c:c+1])
            # h-LIF over t; dw-half spikes written into padded tile, gate into sp2
            m_h = mh_p.tile([P, 512], F32, name=f"m_h{ch}", tag="t")
            sp2 = sp2_p.tile([P, 1024], BF16, name=f"sp2_{ch}", tag="t")
            xp1 = xp1_bufs[ch % 2]
            xh3 = xh.rearrange("p (h q) -> p h q", h=2)
            mh3 = m_h.rearrange("p (h q) -> p h q", h=2)
            xh4 = xh.rearrange("p (a h w) -> p a h w", h=16, w=16)
            xp4 = xp1.rearrange("p (t h w) -> p t h w", t=4, h=18, w=18)
            for t in range(T):
                thr = float(2.0 ** t)
                U3 = xh3[:, :, t*N:(t+1)*N]
                if t > 0:
                    GE.tensor_add(U3, mh3, U3)
                VE.tensor_single_scalar(xp4[:, t, 1:17, 1:17], xh4[:, t], thr, AL.is_ge)
                VE.tensor_single_scalar(sp2[:, t*N:(t+1)*N],
                                        xh[:, 1024 + t*N: 1024 + (t+1)*N], thr, AL.is_ge)
                if t < T - 1:
                    VE.scalar_tensor_tensor(out=mh3, in0=U3, scalar=thr, in1=U3,
                                            op0=AL.is_lt, op1=AL.mult)
            # depthwise conv on the PE: diag(bias) @ ones + sum diag(tap) @ shifted
            acc = acc_p.tile([P, 1024], F32, name=f"acc{ch}", tag="t")
            for tb in range(2):
                pa = ps_dw.tile([P, 512], F32, name=f"dwa{ch}{tb}", tag="pdw")
                for j, (dy, dx) in enumerate(TAPS):
                    rhs = xp4[:, tb*2:(tb+1)*2, dy:dy+16, dx:dx+16]
                    nc.tensor.matmul(pa, dwd[:, (ch*9+j)*P:(ch*9+j+1)*P], rhs,
                                     start=(j == 0), stop=(j == 8))
                nc.scalar.activation(acc[:, tb*512:(tb+1)*512], pa, AF.Identity,
                                     bias=bdw[:, ch:ch+1], scale=1.0)
            # dw-LIF + gate -> mg0 (t-pair 0) / mg1 (t-pair 1), bf16
            m_dw = mdw_p.tile([P, N], F32, name=f"m_dw{ch}", tag="t")
            mg0 = mg0_p.tile([P, 512], BF16, name=f"mg0_{ch}", tag="t")
            mg1 = mg1_p.tile([P, 512], BF16, name=f"mg1_{ch}", tag="t")
            for t in range(T):
                thr = float(2.0 ** t)
                mg = mg0 if t < 2 else mg1
                U = acc[:, t*N:(t+1)*N]
                if t > 0:
                    VE.scalar_tensor_tensor(out=U, in0=U, scalar=thr, in1=m_dw,
                                            op0=AL.mult, op1=AL.add)
                VE.scalar_tensor_tensor(out=mg[:, (t % 2)*N:(t % 2 + 1)*N], in0=U,
                                        scalar=thr, in1=sp2[:, t*N:(t+1)*N],
                                        op0=AL.is_ge, op1=AL.mult)
                if t < T - 1:
                    VE.scalar_tensor_tensor(out=m_dw, in0=U, scalar=thr, in1=U,
                                            op0=AL.is_lt, op1=AL.mult)
            mg_t.append(mg1)
            # fc2 t-pair 0 accumulate
            for mt in range(3):
                nc.tensor.matmul(po1[mt], f2_w[ch][:, mt*P:(mt+1)*P],
                                 mg0,
                                 start=(ch == 0), stop=(ch == 7), skip_group_check=True)

        # fc2 t-pair 1 tail
        po2 = [ps_po.tile([P, 512], F32, name=f"po2_{mt}", tag="po")
               for mt in range(3)]
        for mt in range(3):
            for ch in range(8):
                nc.tensor.matmul(po2[mt], f2_w[ch][:, mt*P:(mt+1)*P],
                                 mg_t[ch],
                                 start=(ch == 0), stop=(ch == 7), skip_group_check=True)

        # fc2 evict + final LIF + residual + store
        xo_t = [xo_p.tile([P, 768], F32, name=f"xo{t}", tag="t") for t in range(T)]
        for t in range(T):
            po = po1 if t < 2 else po2
            for mt in range(3):
                c = mt * 4 + t
                nc.scalar.activation(xo_t[t][:, mt*N:(mt+1)*N],
                                     po[mt][:, (t % 2)*N:(t % 2 + 1)*N],
                                     AF.Identity, bias=b_o[:, c:c+1], scale=a_o[:, c:c+1])
        for t in range(T):
            thr = float(2.0 ** t)
            for mt in range(3):
                U = xo_t[t][:, mt*N:(mt+1)*N]
                if t > 0:
                    GE.tensor_add(U, m_o[mt], U)
                if t < T - 1:
                    VE.scalar_tensor_tensor(out=m_o[mt], in0=U, scalar=thr, in1=U,
                                            op0=AL.is_lt, op1=AL.mult)
                # final out in place over xo (reset already consumed U)
                VE.scalar_tensor_tensor(
                    out=U, in0=U, scalar=thr,
                    in1=xs_kt[mt][:, t*N:(t+1)*N], op0=AL.is_ge, op1=AL.add)
                nc.sync.dma_start(out=d['out'][t*C + mt*P: t*C + (mt+1)*P, :],
                                  in_=U)


def _build():
    nc = bacc.Bacc()
    with tile.TileContext(nc) as tc:
        with tc.tile_pool(name="dram", bufs=1, space="DRAM") as dram:
            def din(name, shape, dt=F32):
                return dram.tile(shape, dt, kind="ExternalInput", name=name,
                                 uniquify=False)
            d = {
                'xin': din('xin', [C, 4 * N]),
                'out': dram.tile([T * C, N], F32, kind="ExternalOutput",
                                 name='out', uniquify=False),
                'q_w': din('q_w', [384, 384], BF16),
                's_q': din('s_q', [ED, 32]),
                'vk_w': din('vk_w', [384, 484], BF16),
                'rb2': din('rb2', [1, 484], BF16),
                'ones': din('ones', [1, 128], BF16),
                'pj_w': din('pj_w', [384, 384], BF16),
                's_po': din('s_po', [128, 48]),
                'f1_w': din('f1_w', [384, 2048], BF16),
                's_h': din('s_h', [128, 128]),
                'dwd': din('dwd', [128, 72 * 128], BF16),
                'b_dw': din('b_dw', [128, 8]),
                'f2_w': din('f2_w', [1024, 384], BF16),
                'ident': din('ident', [128, 128], BF16),
            }
            _body(nc, tc, d)
    nc.finalize()
    return nc


_NC_CACHE = {}


def _get_nc():
    if 'nc' not in _NC_CACHE:
        _NC_CACHE['nc'] = _build()
    return _NC_CACHE['nc']


def _tcols(a):
    rows, k = a.shape
    out = np.empty((rows, k * 4), np.float32)
    for u in range(k):
        for t in range(4):
            out[:, u * 4 + t] = a[:, u] * (2.0 ** t)
    return out


def _prep_common(inputs):
    inp = {k: np.asarray(v, np.float32) for k, v in inputs.items()}
    bf16 = ml_dtypes.bfloat16

    exp_wT = np.concatenate([inp['exp_w'][e].T for e in range(NE)], axis=1)
    a_q = np.zeros((ED, NE), np.float32)
    b_q = np.zeros((ED, NE), np.float32)
    for e in range(NE):
        a_q[:, e] = 0.5 * inp['exp_g'][e] * S
        b_q[:, e] = 0.5 * inp['exp_b'][e]
    rw = inp['router_w'].T * (inp['router_g'] * S)[None, :]
    vk = np.concatenate([inp['v_w'].T, inp['k_w'].T, rw], axis=1)
    rb2 = np.zeros((1, 484), np.float32)
    rb2[0, 480:484] = (inp['router_b'] * inp['router_g'] * S + inp['router_be'])

    g = inp['dw_g']
    taps = (inp['dw_w'][:, 0] * (0.5 * g * S)[:, None, None]).reshape(HH, 9)
    bias = 0.5 * (inp['dw_b'] * g * S + inp['dw_be'])
    dwd = np.zeros((P, 72, P), np.float32)
    pi = np.arange(P)
    for ch in range(8):
        cg = ch * P + pi
        for j in range(9):
            dwd[pi, ch*9+j, pi] = taps[cg, j]

    com = {
        'q_w': exp_wT.astype(bf16),
        's_q': np.concatenate([_tcols(a_q), _tcols(b_q)], axis=1),
        'vk_w': vk.astype(bf16),
        'rb2': rb2.astype(bf16),
        'ones': np.ones((1, 128), bf16),
        'pj_w': inp['proj_w'].T.astype(bf16),
        's_po': np.concatenate([
            _tcols((0.5 * inp['proj_g'] * S).reshape(3, 128).T),
            _tcols((0.5 * (inp['proj_b'] * inp['proj_g'] * S
                           + inp['proj_be'])).reshape(3, 128).T),
            _tcols((0.5 * inp['fc2_g'] * S).reshape(3, 128).T),
            _tcols((0.5 * (inp['fc2_b'] * inp['fc2_g'] * S
                           + inp['fc2_be'])).reshape(3, 128).T)], axis=1),
        'f1_w': inp['fc1_w'].T.astype(bf16),
        's_h': np.concatenate([
            _tcols((0.5 * inp['fc1_g'] * S).reshape(16, 128).T),
            _tcols((0.5 * (inp['fc1_b'] * inp['fc1_g'] * S
                           + inp['fc1_be'])).reshape(16, 128).T)], axis=1),
        'dwd': dwd.reshape(P, 72 * P).astype(bf16),
        'b_dw': np.ascontiguousarray(bias.reshape(8, P).T),
        'f2_w': inp['fc2_w'].T.astype(bf16),
        'ident': np.eye(128, dtype=bf16),
    }
    return {k: np.ascontiguousarray(v) for k, v in com.items()}


def run(inputs, trace=False, tmpdir=None):
    com = _prep_common(inputs)
    x = np.asarray(inputs['x'], np.float32).reshape(T, B, C, N)
    in_maps = []
    for b in range(B):
        m = dict(com)
        m['xin'] = np.ascontiguousarray(x[:, b].transpose(1, 0, 2).reshape(C, T * N))
        in_maps.append(m)
    res = run_bass_kernel_spmd(_get_nc(), in_maps, list(range(B)),
                               trace=trace, tmpdir=tmpdir)
    out = np.empty((T, B, C, N), np.float32)
    for b in range(B):
        out[:, b] = res.results[b]['out'].reshape(T, C, N)
    return out.reshape(T * B, C, 16, 16), res.exec_time_ns


def kernel(**inputs):
    out, _ = run(inputs)
    return out


# revision 17
# speedup vs baseline: 1.6441x; 1.0445x over previous
"""Trainium2 Bass kernel for nn_Block_31954556682442 (spiking MoE-SSA block).

Sharding: pure data-parallel over batch B=8 -> one sample (4 LIF time steps)
per NeuronCore, zero collectives. v3 design:
  - ALL weight matmuls single-term bf16 (W and x both bf16-rounded); CPU-sim
    rel err 4.4e-3 vs the 2e-2 gate
  - kv-first attention: res_e = q_e^T (k v^T); kv integers <=256 exact bf16;
    k^T computed in v's layout by widening the v matmul rhs (no transposes)
  - attention-path LIF state kept in bf16 (values quantized to 0.5 and
    compare-safe), enabling 2x/4x DVE modes
  - depthwise 3x3 conv moved to the PE: per-(ch,tap) diagonal bf16 weights
    x zero-padded spike tiles; bias via diag @ ones
  - fc2 split: t-pair 0 accumulated across the ch loop (3 PSUM banks),
    t-pair 1 as a tail overlapped with the final LIF
  - LIF scans in 2^t-scaled form as in v2
Self-contained: hardcodes all shapes; no sibling imports.
"""
import numpy as np
import ml_dtypes

import concourse.bacc as bacc
import concourse.mybir as mybir
import concourse.tile as tile
from concourse.bass_utils import run_bass_kernel_spmd

F32 = mybir.dt.float32
BF16 = mybir.dt.bfloat16
AL = mybir.AluOpType
AF = mybir.ActivationFunctionType

T, B, C, N = 4, 8, 384, 256
ED = 96
NE = 4
HID, HH = 2048, 1024
S = float(1.0 / np.sqrt(1.0 + 1e-5))
P = 128


def _body(nc, tc, d):
    from contextlib import ExitStack
    VE = nc.vector
    GE = nc.gpsimd

    with ExitStack() as ctx:
        def pool(name, bufs, space="SBUF"):
            return ctx.enter_context(tc.tile_pool(name=name, bufs=bufs, space=space))

        wp = pool("wp", 1)
        mp = pool("mp", 1)
        xs_p = pool("xs_p", 3)       # (128,1024) f32 x, doubles as x_new
        xq_p = pool("xq_p", 3)       # (128,1024) bf16 rounded x
        xqt_p = pool("xqt_p", 3)     # (96,1024) f32 q pre-act
        xvt_p = pool("xvt_p", 3)     # (128,960) f32 v|k pre-act
        xrt_p = pool("xrt_p", 2)     # (128,8)
        spq_p = pool("spq_p", 4)     # (96,1024) bf16 q spikes
        vks_p = pool("vks_p", 4)     # (128,960) bf16 v|k spikes
        wsp_p = pool("wsp_p", 4)     # (128,8) f32 router spikes
        kv_p = pool("kv_p", 2)       # (96,384) bf16
        xr_p = pool("xr_p", 3)       # (128,768) bf16 res pre-act
        rs_p = pool("rs_p", 2)       # (128,768) bf16 masked res spikes
        y_p = pool("y_p", 4)         # (128,768) bf16
        ydn_p = pool("ydn_p", 3)     # (128,1024) bf16
        xp_p = pool("xp_p", 2)       # (128,768) f32 proj pre-act
        xh_p = pool("xh_p", 2)       # (128,2048) f32
        sp2_p = pool("sp2_p", 2)     # (128,1024) bf16 gate spikes
        xp1_p = pool("xp1_p", 1)     # padded dw spikes, 2 tiles
        acc_p = pool("acc_p", 2)     # (128,1024) f32 dw-conv acc
        mg0_p = pool("mg0_p", 2)     # (128,512) bf16 t-pair0 gated spikes
        mg1_p = pool("mg1_p", 8)     # (128,512) bf16 t-pair1 gated spikes
        mh_p = pool("mh_p", 2)       # (128,512) f32
        mdw_p = pool("mdw_p", 1)     # (128,256) f32
        xo_p = pool("xo_p", 2)       # (128,768) f32

        psum_ad = ExitStack()
        ps_ad = psum_ad.enter_context(
            tc.tile_pool(name="ps_ad", bufs=6, space="PSUM"))

        # ---------------- weight loads ----------------
        def wload(name, shape, dt=F32, src=None):
            w = wp.tile(shape, dt, name=name, tag=name)
            nc.sync.dma_start(out=w, in_=d[name] if src is None else src)
            return w

        ident = wload('ident', [P, P], BF16)
        # PE warmup: dummy matmuls to flip HAM to K=8/8 before phase A
        pwarm = ps_ad.tile([P, P], F32, name="pwarm", tag="pm")
        for wi in range(24):
            nc.tensor.matmul(pwarm, ident, ident, start=True, stop=True)
        warm_sink = wp.tile([P, 1], F32, name="warm_sink", tag="warm_sink")
        nc.scalar.activation(warm_sink, pwarm[:, 0:1], AF.Copy)

        # xs first (phase A starts on these)
        xs_kt = []
        for kt in range(3):
            x_ = xs_p.tile([P, 4 * N], F32, name=f"xs{kt}", tag="t")
            xs_kt.append(x_)
        for kt in range(3):
            nc.sync.dma_start(out=xs_kt[kt], in_=d['xin'][kt*P:(kt+1)*P, :])
        q_w, vk_w, pj_w, f1_w, f2_w = [], [], [], [], []
        for kt in range(3):
            q_w.append(wload(f'q_w{kt}', [P, 384], BF16, d['q_w'][kt*P:(kt+1)*P, :]))
            vk_w.append(wload(f'vk_w{kt}', [P, 484], BF16, d['vk_w'][kt*P:(kt+1)*P, :]))
        sq = wload('s_q', [ED, 32])
        a_q, b_q = sq[:, 0:16], sq[:, 16:32]
        rb2 = wload('rb2', [1, 484], BF16); ones = wload('ones', [1, P], BF16)
        for kt in range(3):
            pj_w.append(wload(f'pj_w{kt}', [P, 384], BF16, d['pj_w'][kt*P:(kt+1)*P, :]))
        spo = wload('s_po', [P, 48])
        a_p, b_p = spo[:, 0:12], spo[:, 12:24]
        a_o, b_o = spo[:, 24:36], spo[:, 36:48]
        for kt in range(3):
            f1_w.append(wload(f'f1_w{kt}', [P, 2048], BF16, d['f1_w'][kt*P:(kt+1)*P, :]))
        sh = wload('s_h', [P, 128])
        a_h, b_h = sh[:, 0:64], sh[:, 64:128]
        dwd = wload('dwd', [P, 72 * P], BF16)
        bdw = wload('b_dw', [P, 8])
        for ch in range(8):
            f2_w.append(wload(f'f2_w{ch}', [P, 384], BF16, d['f2_w'][ch*P:(ch+1)*P, :]))

        # ---------------- x -> bf16 ----------------
        xq_kt = []
        for kt in range(3):
            q_ = xq_p.tile([P, 4 * N], BF16, name=f"xq{kt}", tag="t")
            nc.scalar.activation(q_, xs_kt[kt], AF.Copy)
            xq_kt.append(q_)

        # ---------------- phase A: q / v|k / router matmuls + evicts ----------------
        m_kq = mp.tile([ED, 4 * N], F32, name="m_kq", tag="m_kq")
        m_vt = mp.tile([P, 968], F32, name="m_vt", tag="m_vt")
        m_p = mp.tile([P, 768], F32, name="m_p", tag="m_p")
        m_o = [mp.tile([P, N], F32, name=f"m_o{i}", tag=f"m_o{i}") for i in range(3)]

        xq_t = [xqt_p.tile([ED, 4 * N], F32, name=f"xqt{t}", tag="t") for t in range(T)]
        xvt_t = [xvt_p.tile([P, 968], F32, name=f"xvt{t}", tag="t") for t in range(T)]

        for tp in range(2):
            for u in range(NE):
                pt = ps_ad.tile([ED, 512], F32, name=f"pq{u}_{tp}", tag="pm")
                for kt in range(3):
                    nc.tensor.matmul(pt, q_w[kt][:, 96*u:96*(u+1)],
                                     xq_kt[kt][:, tp*512:(tp+1)*512],
                                     start=(kt == 0), stop=(kt == 2))
                for ti in range(2):
                    t = tp * 2 + ti
                    c = u * 4 + t
                    nc.scalar.activation(xq_t[t][:, u*N:(u+1)*N], pt[:, ti*N:(ti+1)*N],
                                         AF.Identity, bias=b_q[:, c:c+1], scale=a_q[:, c:c+1])
        for t in range(T):
            for mt in range(2):
                pv = ps_ad.tile([P, 484], F32, name=f"pvt{t}_{mt}", tag="pm")
                for kt in range(3):
                    nc.tensor.matmul(pv, xq_kt[kt][:, t*N + mt*P: t*N + (mt+1)*P],
                                     vk_w[kt], start=(kt == 0), stop=False,
                                     skip_group_check=True)
                nc.tensor.matmul(pv[:, 480:484], ones, rb2[:, 480:484],
                                 start=False, stop=True, skip_group_check=True)
                nc.scalar.activation(xvt_t[t][:, mt*484:(mt+1)*484], pv, AF.Copy,
                                     bias=0.0, scale=0.5 * float(2.0 ** t))

        # ---------------- phase B: LIF scans for q / v|k / r ----------------
        sp_q, vk_sp, w_sp = [], [], []
        for t in range(T):
            thr = float(2.0 ** t)
            U = xq_t[t]
            if t > 0:
                GE.tensor_add(U, m_kq, U)
            sp = spq_p.tile([ED, 4 * N], BF16, name=f"spq{t}", tag="t")
            VE.tensor_single_scalar(sp, U, thr, AL.is_ge)
            if t < T - 1:
                VE.scalar_tensor_tensor(out=m_kq, in0=U, scalar=thr, in1=U,
                                        op0=AL.is_lt, op1=AL.mult)
            sp_q.append(sp)

            U = xvt_t[t]
            if t > 0:
                GE.tensor_add(U, m_vt, U)
            vs = vks_p.tile([P, 968], BF16, name=f"vks{t}", tag="t")
            VE.tensor_single_scalar(vs, U, thr, AL.is_ge)
            # router spike columns as f32 (tensor_scalar scalar2 operand)
            ws = wsp_p.tile([P, 8], F32, name=f"wsp{t}", tag="t")
            U3 = U.rearrange("p (m c) -> p m c", m=2)
            VE.tensor_single_scalar(ws.rearrange("p (m c) -> p m c", m=2),
                                    U3[:, :, 480:484], thr, AL.is_ge)
            w_sp.append(ws)
            if t < T - 1:
                VE.scalar_tensor_tensor(out=m_vt, in0=U, scalar=thr, in1=U,
                                        op0=AL.is_lt, op1=AL.mult)
            vk_sp.append(vs)
            # keep-warm: tiny matmul chained on this step's spikes so the PE
            # HAM window never sees a fully idle interval during the LIF scan
            nc.tensor.matmul(pwarm, ident, vs[:, 0:P], start=True, stop=True)

        # ---------------- phase C: kv-first experts ----------------
        # masked spikes: w_e*(U>=thr) via one tensor_scalar (4x mode);
        # y accumulated with plain bf16 tensor_tensor adds (2x mode)
        y = [None] * T
        m_res_e = [mp.tile([P, 768], BF16, name=f"m_res{e}", tag=f"m_res{e}")
                   for e in range(NE)]
        for t in range(T):
            thr = float(2.0 ** t)
            pkv = ps_ad.tile([ED, 384], F32, name=f"pkv{t}", tag="pm")
            for mt in range(2):
                nc.tensor.matmul(pkv, vk_sp[t][:, mt*484+384:mt*484+480],
                                 vk_sp[t][:, mt*484:mt*484+384],
                                 start=(mt == 0), stop=(mt == 1))
            kv = kv_p.tile([ED, 384], BF16, name=f"kv{t}", tag="t")
            nc.scalar.activation(kv, pkv, AF.Copy)
            yt = y_p.tile([P, 768], BF16, name=f"y{t}", tag="t")
            y[t] = yt
            for e in range(NE):
                m_res = m_res_e[e]
                xr = xr_p.tile([P, 768], BF16, name=f"xres{e}{t}", tag="t")
                for mt in range(2):
                    pr_ = ps_ad.tile([P, 384], F32, name=f"pres{e}{t}{mt}", tag="pm")
                    nc.tensor.matmul(pr_, sp_q[t][:, e*N + mt*P: e*N + (mt+1)*P],
                                     kv, start=True, stop=True)
                    nc.scalar.activation(xr[:, mt*384:(mt+1)*384], pr_, AF.Copy,
                                         bias=0.0, scale=0.5 * thr)
                U = xr
                if t > 0:
                    VE.tensor_add(U, m_res, U)
                dst = yt if e == 0 else rs_p.tile([P, 768], BF16,
                                                  name=f"rsm{e}{t}", tag="t")
                for mt in range(2):
                    VE.tensor_scalar(out=dst[:, mt*384:(mt+1)*384],
                                     in0=U[:, mt*384:(mt+1)*384],
                                     scalar1=thr,
                                     scalar2=w_sp[t][:, mt*4+e:mt*4+e+1],
                                     op0=AL.is_ge, op1=AL.mult)
                if t < T - 1:
                    VE.scalar_tensor_tensor(out=m_res, in0=U, scalar=thr, in1=U,
                                            op0=AL.is_lt, op1=AL.mult)
                if e > 0:
                    VE.tensor_add(yt, yt, dst)
                nc.tensor.matmul(pwarm, ident, U[:, 0:P], start=True, stop=True)

        # ---------------- phase D: transpose y, proj, LIF, residual ----------------
        ydn = [ydn_p.tile([P, 4 * N], BF16, name=f"ydn{dt}", tag="t") for dt in range(3)]
        xp_t = [xp_p.tile([P, 768], F32, name=f"xp{t}", tag="t") for t in range(T)]
        for tp in range(2):
            for t in (tp * 2, tp * 2 + 1):
                for mt in range(2):
                    for dt in range(3):
                        ptr = ps_ad.tile([P, P], BF16, name=f"ptr{t}{mt}{dt}", tag="pm")
                        nc.tensor.transpose(
                            ptr, y[t][:, mt*384 + dt*P: mt*384 + (dt+1)*P], ident)
                        nc.scalar.activation(ydn[dt][:, t*N + mt*P: t*N + (mt+1)*P],
                                             ptr, AF.Copy)
            for mt in range(3):
                pp = ps_ad.tile([P, 512], F32, name=f"pp{mt}_{tp}", tag="pm")
                for kt in range(3):
                    nc.tensor.matmul(pp, pj_w[kt][:, mt*P:(mt+1)*P],
                                     ydn[kt][:, tp*512:(tp+1)*512],
                                     start=(kt == 0), stop=(kt == 2))
                for ti in range(2):
                    t = tp * 2 + ti
                    c = mt * 4 + t
                    nc.scalar.activation(xp_t[t][:, mt*N:(mt+1)*N], pp[:, ti*N:(ti+1)*N],
                                         AF.Identity, bias=b_p[:, c:c+1], scale=a_p[:, c:c+1])
            for t in (tp * 2, tp * 2 + 1):
                thr = float(2.0 ** t)
                U = xp_t[t]
                if t > 0:
                    GE.tensor_add(U, m_p, U)
                if t < T - 1:
                    VE.scalar_tensor_tensor(out=m_p, in0=U, scalar=thr, in1=U,
                                            op0=AL.is_lt, op1=AL.mult)
                for mt in range(3):
                    # x_new overwrites xs in place (residual add)
                    VE.scalar_tensor_tensor(
                        out=xs_kt[mt][:, t*N:(t+1)*N], in0=U[:, mt*N:(mt+1)*N],
                        scalar=thr, in1=xs_kt[mt][:, t*N:(t+1)*N],
                        op0=AL.is_ge, op1=AL.add)
            # x_new -> bf16 for this time-pair (fc1 can start on tp=0)
            for kt in range(3):
                nc.scalar.activation(xq_kt[kt][:, tp*512:(tp+1)*512],
                                     xs_kt[kt][:, tp*512:(tp+1)*512], AF.Copy)

        # ---------------- phase E: MLP ----------------
        psum_ad.close()
        ps_e = ctx.enter_context(tc.tile_pool(name="ps_e", bufs=3, space="PSUM"))
        ps_dw = ctx.enter_context(tc.tile_pool(name="ps_dw", bufs=2, space="PSUM"))
        ps_po = ctx.enter_context(tc.tile_pool(name="ps_po", bufs=3, space="PSUM"))
        # padded dw-spike tiles (zero borders written once)
        xp1_bufs = [xp1_p.tile([P, 1296], BF16, name=f"xp1_{i}", tag=f"xp1_{i}")
                    for i in range(2)]
        for b_ in xp1_bufs:
            GE.memset(b_, 0.0)
        po1 = [ps_po.tile([P, 512], F32, name=f"po1_{mt}", tag="po")
               for mt in range(3)]
        TAPS = [(dy, dx) for dy in range(3) for dx in range(3)]
        mg_t = []
        for ch in range(8):
            xh = xh_p.tile([P, 2048], F32, name=f"xh{ch}", tag="t")
            for half in range(2):
                mth = ch + 8 * half
                for tp in range(2):
                    ph = ps_e.tile([P, 512], F32, name=f"ph{ch}{half}{tp}", tag="pm")
                    for kt in range(3):
                        nc.tensor.matmul(ph, f1_w[kt][:, mth*P:(mth+1)*P],
                                         xq_kt[kt][:, tp*512:(tp+1)*512],
                                         start=(kt == 0), stop=(kt == 2))
                    for ti in range(2):
                        t = tp * 2 + ti
                        c = mth * 4 + t
                        nc.scalar.activation(
                            xh[:, half*1024 + t*N: half*1024 + (t+1)*N],
                            ph[:, ti*N:(ti+1)*N], AF.Identity,
                            bias=b_h[:, c:c+1], scale=a_h[:, c:c+1])
            # h-LIF over t; dw-half spikes written into padded tile, gate into sp2
            m_h = mh_p.tile([P, 512], F32, name=f"m_h{ch}", tag="t")
            sp2 = sp2_p.tile([P, 1024], BF16, name=f"sp2_{ch}", tag="t")
            xp1 = xp1_bufs[ch % 2]
            xh3 = xh.rearrange("p (h q) -> p h q", h=2)
            mh3 = m_h.rearrange("p (h q) -> p h q", h=2)
            xh4 = xh.rearrange("p (a h w) -> p a h w", h=16, w=16)
            xp4 = xp1.rearrange("p (t h w) -> p t h w", t=4, h=18, w=18)
            for t in range(T):
                thr = float(2.0 ** t)
                U3 = xh3[:, :, t*N:(t+1)*N]
                if t > 0:
                    GE.tensor_add(U3, mh3, U3)
                VE.tensor_single_scalar(xp4[:, t, 1:17, 1:17], xh4[:, t], thr, AL.is_ge)
                VE.tensor_single_scalar(sp2[:, t*N:(t+1)*N],
                                        xh[:, 1024 + t*N: 1024 + (t+1)*N], thr, AL.is_ge)
                if t < T - 1:
                    VE.scalar_tensor_tensor(out=mh3, in0=U3, scalar=thr, in1=U3,
                                            op0=AL.is_lt, op1=AL.mult)
            # depthwise conv on the PE: diag(bias) @ ones + sum diag(tap) @ shifted
            acc = acc_p.tile([P, 1024], F32, name=f"acc{ch}", tag="t")
            for tb in range(2):
                pa = ps_dw.tile([P, 512], F32, name=f"dwa{ch}{tb}", tag="pdw")
                for j, (dy, dx) in enumerate(TAPS):
                    rhs = xp4[:, tb*2:(tb+1)*2, dy:dy+16, dx:dx+16]
                    nc.tensor.matmul(pa, dwd[:, (ch*9+j)*P:(ch*9+j+1)*P], rhs,
                                     start=(j == 0), stop=(j == 8))
                nc.scalar.activation(acc[:, tb*512:(tb+1)*512], pa, AF.Identity,
                                     bias=bdw[:, ch:ch+1], scale=1.0)
            # dw-LIF + gate -> mg0 (t-pair 0) / mg1 (t-pair 1), bf16
            m_dw = mdw_p.tile([P, N], F32, name=f"m_dw{ch}", tag="t")
            mg0 = mg0_p.tile([P, 512], BF16, name=f"mg0_{ch}", tag="t")
            mg1 = mg1_p.tile([P, 512], BF16, name=f"mg1_{ch}", tag="t")
            for t in range(T):
                thr = float(2.0 ** t)
                mg = mg0 if t < 2 else mg1
                U = acc[:, t*N:(t+1)*N]
                if t > 0:
                    VE.scalar_tensor_tensor(out=U, in0=U, scalar=thr, in1=m_dw,
                                            op0=AL.mult, op1=AL.add)
                VE.scalar_tensor_tensor(out=mg[:, (t % 2)*N:(t % 2 + 1)*N], in0=U,
                                        scalar=thr, in1=sp2[:, t*N:(t+1)*N],
                                        op0=AL.is_ge, op1=AL.mult)
                if t < T - 1:
                    VE.scalar_tensor_tensor(out=m_dw, in0=U, scalar=thr, in1=U,
                                            op0=AL.is_lt, op1=AL.mult)
            mg_t.append(mg1)
            # fc2 t-pair 0 accumulate
            for mt in range(3):
                nc.tensor.matmul(po1[mt], f2_w[ch][:, mt*P:(mt+1)*P],
                                 mg0,
                                 start=(ch == 0), stop=(ch == 7), skip_group_check=True)

        # fc2 t-pair 1 tail
        po2 = [ps_po.tile([P, 512], F32, name=f"po2_{mt}", tag="po")
               for mt in range(3)]
        for mt in range(3):
            for ch in range(8):
                nc.tensor.matmul(po2[mt], f2_w[ch][:, mt*P:(mt+1)*P],
                                 mg_t[ch],
                                 start=(ch == 0), stop=(ch == 7), skip_group_check=True)

        # fc2 evict + final LIF + residual + store
        xo_t = [xo_p.tile([P, 768], F32, name=f"xo{t}", tag="t") for t in range(T)]
        for t in range(T):
            po = po1 if t < 2 else po2
            for mt in range(3):
                c = mt * 4 + t
                nc.scalar.activation(xo_t[t][:, mt*N:(mt+1)*N],
                                     po[mt][:, (t % 2)*N:(t % 2 + 1)*N],
                                     AF.Identity, bias=b_o[:, c:c+1], scale=a_o[:, c:c+1])
        for t in range(T):
            thr = float(2.0 ** t)
            for mt in range(3):
                U = xo_t[t][:, mt*N:(mt+1)*N]
                if t > 0:
                    GE.tensor_add(U, m_o[mt], U)
                if t < T - 1:
                    VE.scalar_tensor_tensor(out=m_o[mt], in0=U, scalar=thr, in1=U,
                                            op0=AL.is_lt, op1=AL.mult)
                # final out in place over xo (reset already consumed U)
                VE.scalar_tensor_tensor(
                    out=U, in0=U, scalar=thr,
                    in1=xs_kt[mt][:, t*N:(t+1)*N], op0=AL.is_ge, op1=AL.add)
                nc.sync.dma_start(out=d['out'][t*C + mt*P: t*C + (mt+1)*P, :],
                                  in_=U)


def _build():
    nc = bacc.Bacc()
    with tile.TileContext(nc) as tc:
        with tc.tile_pool(name="dram", bufs=1, space="DRAM") as dram:
            def din(name, shape, dt=F32):
                return dram.tile(shape, dt, kind="ExternalInput", name=name,
                                 uniquify=False)
            d = {
                'xin': din('xin', [C, 4 * N]),
                'out': dram.tile([T * C, N], F32, kind="ExternalOutput",
                                 name='out', uniquify=False),
                'q_w': din('q_w', [384, 384], BF16),
                's_q': din('s_q', [ED, 32]),
                'vk_w': din('vk_w', [384, 484], BF16),
                'rb2': din('rb2', [1, 484], BF16),
                'ones': din('ones', [1, 128], BF16),
                'pj_w': din('pj_w', [384, 384], BF16),
                's_po': din('s_po', [128, 48]),
                'f1_w': din('f1_w', [384, 2048], BF16),
                's_h': din('s_h', [128, 128]),
                'dwd': din('dwd', [128, 72 * 128], BF16),
                'b_dw': din('b_dw', [128, 8]),
                'f2_w': din('f2_w', [1024, 384], BF16),
                'ident': din('ident', [128, 128], BF16),
            }
            _body(nc, tc, d)
    nc.finalize()
    return nc


_NC_CACHE = {}


def _get_nc():
    if 'nc' not in _NC_CACHE:
        _NC_CACHE['nc'] = _build()
    return _NC_CACHE['nc']


def _tcols(a):
    rows, k = a.shape
    out = np.empty((rows, k * 4), np.float32)
    for u in range(k):
        for t in range(4):
            out[:, u * 4 + t] = a[:, u] * (2.0 ** t)
    return out


def _prep_common(inputs):
    inp = {k: np.asarray(v, np.float32) for k, v in inputs.items()}
    bf16 = ml_dtypes.bfloat16

    exp_wT = np.concatenate([inp['exp_w'][e].T for e in range(NE)], axis=1)
    a_q = np.zeros((ED, NE), np.float32)
    b_q = np.zeros((ED, NE), np.float32)
    for e in range(NE):
        a_q[:, e] = 0.5 * inp['exp_g'][e] * S
        b_q[:, e] = 0.5 * inp['exp_b'][e]
    rw = inp['router_w'].T * (inp['router_g'] * S)[None, :]
    vk = np.concatenate([inp['v_w'].T, inp['k_w'].T, rw], axis=1)
    rb2 = np.zeros((1, 484), np.float32)
    rb2[0, 480:484] = (inp['router_b'] * inp['router_g'] * S + inp['router_be'])

    g = inp['dw_g']
    taps = (inp['dw_w'][:, 0] * (0.5 * g * S)[:, None, None]).reshape(HH, 9)
    bias = 0.5 * (inp['dw_b'] * g * S + inp['dw_be'])
    dwd = np.zeros((P, 72, P), np.float32)
    pi = np.arange(P)
    for ch in range(8):
        cg = ch * P + pi
        for j in range(9):
            dwd[pi, ch*9+j, pi] = taps[cg, j]

    com = {
        'q_w': exp_wT.astype(bf16),
        's_q': np.concatenate([_tcols(a_q), _tcols(b_q)], axis=1),
        'vk_w': vk.astype(bf16),
        'rb2': rb2.astype(bf16),
        'ones': np.ones((1, 128), bf16),
        'pj_w': inp['proj_w'].T.astype(bf16),
        's_po': np.concatenate([
            _tcols((0.5 * inp['proj_g'] * S).reshape(3, 128).T),
            _tcols((0.5 * (inp['proj_b'] * inp['proj_g'] * S
                           + inp['proj_be'])).reshape(3, 128).T),
            _tcols((0.5 * inp['fc2_g'] * S).reshape(3, 128).T),
            _tcols((0.5 * (inp['fc2_b'] * inp['fc2_g'] * S
                           + inp['fc2_be'])).reshape(3, 128).T)], axis=1),
        'f1_w': inp['fc1_w'].T.astype(bf16),
        's_h': np.concatenate([
            _tcols((0.5 * inp['fc1_g'] * S).reshape(16, 128).T),
            _tcols((0.5 * (inp['fc1_b'] * inp['fc1_g'] * S
                           + inp['fc1_be'])).reshape(16, 128).T)], axis=1),
        'dwd': dwd.reshape(P, 72 * P).astype(bf16),
        'b_dw': np.ascontiguousarray(bias.reshape(8, P).T),
        'f2_w': inp['fc2_w'].T.astype(bf16),
        'ident': np.eye(128, dtype=bf16),
    }
    return {k: np.ascontiguousarray(v) for k, v in com.items()}


def run(inputs, trace=False, tmpdir=None):
    com = _prep_common(inputs)
    x = np.asarray(inputs['x'], np.float32).reshape(T, B, C, N)
    in_maps = []
    for b in range(B):
        m = dict(com)
        m['xin'] = np.ascontiguousarray(x[:, b].transpose(1, 0, 2).reshape(C, T * N))
        in_maps.append(m)
    res = run_bass_kernel_spmd(_get_nc(), in_maps, list(range(B)),
                               trace=trace, tmpdir=tmpdir)
    out = np.empty((T, B, C, N), np.float32)
    for b in range(B):
        out[:, b] = res.results[b]['out'].reshape(T, C, N)
    return out.reshape(T * B, C, 16, 16), res.exec_time_ns


def kernel(**inputs):
    out, _ = run(inputs)
    return out


# revision 18
# speedup vs baseline: 1.8397x; 1.1190x over previous
"""Trainium2 Bass kernel for nn_Block_31954556682442 (spiking MoE-SSA block).

Sharding: pure data-parallel over batch B=8 -> one sample (4 LIF time steps)
per NeuronCore, zero collectives. v3 design:
  - ALL weight matmuls single-term bf16 (W and x both bf16-rounded); CPU-sim
    rel err 4.4e-3 vs the 2e-2 gate
  - kv-first attention: res_e = q_e^T (k v^T); kv integers <=256 exact bf16;
    k^T computed in v's layout by widening the v matmul rhs (no transposes)
  - attention-path LIF state kept in bf16 (values quantized to 0.5 and
    compare-safe), enabling 2x/4x DVE modes
  - depthwise 3x3 conv moved to the PE: per-(ch,tap) diagonal bf16 weights
    x zero-padded spike tiles; bias via diag @ ones
  - fc2 split: t-pair 0 accumulated across the ch loop (3 PSUM banks),
    t-pair 1 as a tail overlapped with the final LIF
  - LIF scans in 2^t-scaled form as in v2
Self-contained: hardcodes all shapes; no sibling imports.
"""
import numpy as np
import ml_dtypes

import concourse.bacc as bacc
import concourse.mybir as mybir
import concourse.tile as tile
from concourse.bass_utils import run_bass_kernel_spmd

F32 = mybir.dt.float32
BF16 = mybir.dt.bfloat16
AL = mybir.AluOpType
AF = mybir.ActivationFunctionType

T, B, C, N = 4, 8, 384, 256
ED = 96
NE = 4
HID, HH = 2048, 1024
S = float(1.0 / np.sqrt(1.0 + 1e-5))
P = 128


def _body(nc, tc, d):
    from contextlib import ExitStack
    VE = nc.vector
    GE = nc.gpsimd

    with ExitStack() as ctx:
        def pool(name, bufs, space="SBUF"):
            return ctx.enter_context(tc.tile_pool(name=name, bufs=bufs, space=space))

        wp = pool("wp", 1)
        mp = pool("mp", 1)
        xs_p = pool("xs_p", 3)       # (128,1024) f32 x, doubles as x_new
        xq_p = pool("xq_p", 3)       # (128,1024) bf16 rounded x
        xqt_p = pool("xqt_p", 3)     # (96,1024) f32 q pre-act
        xvt_p = pool("xvt_p", 3)     # (128,960) f32 v|k pre-act
        xrt_p = pool("xrt_p", 2)     # (128,8)
        spq_p = pool("spq_p", 4)     # (96,1024) bf16 q spikes
        vks_p = pool("vks_p", 4)     # (128,960) bf16 v|k spikes
        wsp_p = pool("wsp_p", 4)     # (128,8) f32 router spikes
        kv_p = pool("kv_p", 2)       # (96,384) bf16
        xr_p = pool("xr_p", 3)       # (128,768) bf16 res pre-act
        rs_p = pool("rs_p", 2)       # (128,768) bf16 masked res spikes
        y_p = pool("y_p", 4)         # (128,768) bf16
        ydn_p = pool("ydn_p", 3)     # (128,1024) bf16
        xp_p = pool("xp_p", 2)       # (128,768) f32 proj pre-act
        xh_p = pool("xh_p", 2)       # (128,2048) f32
        sp2_p = pool("sp2_p", 2)     # (128,1024) bf16 gate spikes
        xp1_p = pool("xp1_p", 1)     # padded dw spikes, 2 tiles
        acc_p = pool("acc_p", 2)     # (128,1024) f32 dw-conv acc
        mg0_p = pool("mg0_p", 2)     # (128,512) bf16 t-pair0 gated spikes
        mg1_p = pool("mg1_p", 8)     # (128,512) bf16 t-pair1 gated spikes
        mh_p = pool("mh_p", 2)       # (128,512) f32
        mdw_p = pool("mdw_p", 1)     # (128,256) f32
        xo_p = pool("xo_p", 2)       # (128,768) f32

        psum_ad = ExitStack()
        ps_ad = psum_ad.enter_context(
            tc.tile_pool(name="ps_ad", bufs=6, space="PSUM"))

        # ---------------- weight loads ----------------
        def wload(name, shape, dt=F32, src=None):
            w = wp.tile(shape, dt, name=name, tag=name)
            nc.sync.dma_start(out=w, in_=d[name] if src is None else src)
            return w

        ident = wload('ident', [P, P], BF16)
        # PE warmup: dummy matmuls to flip HAM to K=8/8 before phase A
        pwarm = ps_ad.tile([P, P], F32, name="pwarm", tag="pm")
        for wi in range(24):
            nc.tensor.matmul(pwarm, ident, ident, start=True, stop=True)
        warm_sink = wp.tile([P, 1], F32, name="warm_sink", tag="warm_sink")
        nc.scalar.activation(warm_sink, pwarm[:, 0:1], AF.Copy)

        # xs first (phase A starts on these)
        xs_kt = []
        for kt in range(3):
            x_ = xs_p.tile([P, 4 * N], F32, name=f"xs{kt}", tag="t")
            xs_kt.append(x_)
        for kt in range(3):
            nc.sync.dma_start(out=xs_kt[kt], in_=d['xin'][kt*P:(kt+1)*P, :])
        q_w, vk_w, pj_w, f1_w, f2_w = [], [], [], [], []
        for kt in range(3):
            q_w.append(wload(f'q_w{kt}', [P, 384], BF16, d['q_w'][kt*P:(kt+1)*P, :]))
            vk_w.append(wload(f'vk_w{kt}', [P, 484], BF16, d['vk_w'][kt*P:(kt+1)*P, :]))
        sq = wload('s_q', [ED, 32])
        a_q, b_q = sq[:, 0:16], sq[:, 16:32]
        rb2 = wload('rb2', [1, 484], BF16); ones = wload('ones', [1, P], BF16)
        for kt in range(3):
            pj_w.append(wload(f'pj_w{kt}', [P, 384], BF16, d['pj_w'][kt*P:(kt+1)*P, :]))
        spo = wload('s_po', [P, 48])
        a_p, b_p = spo[:, 0:12], spo[:, 12:24]
        a_o, b_o = spo[:, 24:36], spo[:, 36:48]
        for kt in range(3):
            f1_w.append(wload(f'f1_w{kt}', [P, 2048], BF16, d['f1_w'][kt*P:(kt+1)*P, :]))
        sh = wload('s_h', [P, 128])
        a_h, b_h = sh[:, 0:64], sh[:, 64:128]
        dwd = wload('dwd', [P, 72 * P], BF16)
        bdw = wload('b_dw', [P, 8])
        for ch in range(8):
            f2_w.append(wload(f'f2_w{ch}', [P, 384], BF16, d['f2_w'][ch*P:(ch+1)*P, :]))

        # ---------------- x -> bf16 ----------------
        xq_kt = []
        for kt in range(3):
            q_ = xq_p.tile([P, 4 * N], BF16, name=f"xq{kt}", tag="t")
            nc.scalar.activation(q_, xs_kt[kt], AF.Copy)
            xq_kt.append(q_)

        # ---------------- phase A: q / v|k / router matmuls + evicts ----------------
        m_kq = mp.tile([ED, 4 * N], F32, name="m_kq", tag="m_kq")
        m_vt = mp.tile([P, 968], F32, name="m_vt", tag="m_vt")
        m_p = mp.tile([P, 768], F32, name="m_p", tag="m_p")
        m_o = [mp.tile([P, N], F32, name=f"m_o{i}", tag=f"m_o{i}") for i in range(3)]

        xq_t = [xqt_p.tile([ED, 4 * N], F32, name=f"xqt{t}", tag="t") for t in range(T)]
        xvt_t = [xvt_p.tile([P, 968], F32, name=f"xvt{t}", tag="t") for t in range(T)]

        for tp in range(2):
            for u in range(NE):
                pt = ps_ad.tile([ED, 512], F32, name=f"pq{u}_{tp}", tag="pm")
                for kt in range(3):
                    nc.tensor.matmul(pt, q_w[kt][:, 96*u:96*(u+1)],
                                     xq_kt[kt][:, tp*512:(tp+1)*512],
                                     start=(kt == 0), stop=(kt == 2))
                for ti in range(2):
                    t = tp * 2 + ti
                    c = u * 4 + t
                    nc.scalar.activation(xq_t[t][:, u*N:(u+1)*N], pt[:, ti*N:(ti+1)*N],
                                         AF.Identity, bias=b_q[:, c:c+1], scale=a_q[:, c:c+1])
        for t in range(T):
            for mt in range(2):
                pv = ps_ad.tile([P, 484], F32, name=f"pvt{t}_{mt}", tag="pm")
                for kt in range(3):
                    nc.tensor.matmul(pv, xq_kt[kt][:, t*N + mt*P: t*N + (mt+1)*P],
                                     vk_w[kt], start=(kt == 0), stop=False,
                                     skip_group_check=True)
                nc.tensor.matmul(pv[:, 480:484], ones, rb2[:, 480:484],
                                 start=False, stop=True, skip_group_check=True)
                nc.scalar.activation(xvt_t[t][:, mt*484:(mt+1)*484], pv, AF.Copy,
                                     bias=0.0, scale=0.5 * float(2.0 ** t))

        # ---------------- phase B: LIF scans for q / v|k / r ----------------
        sp_q, vk_sp, w_sp = [], [], []
        for t in range(T):
            thr = float(2.0 ** t)
            U = xq_t[t]
            if t > 0:
                GE.tensor_add(U, m_kq, U)
            sp = spq_p.tile([ED, 4 * N], BF16, name=f"spq{t}", tag="t")
            VE.tensor_single_scalar(sp, U, thr, AL.is_ge)
            if t < T - 1:
                VE.scalar_tensor_tensor(out=m_kq, in0=U, scalar=thr, in1=U,
                                        op0=AL.is_lt, op1=AL.mult)
            sp_q.append(sp)

            U = xvt_t[t]
            if t > 0:
                GE.tensor_add(U, m_vt, U)
            vs = vks_p.tile([P, 968], BF16, name=f"vks{t}", tag="t")
            VE.tensor_single_scalar(vs, U, thr, AL.is_ge)
            # router spike columns as f32 (tensor_scalar scalar2 operand)
            ws = wsp_p.tile([P, 8], F32, name=f"wsp{t}", tag="t")
            U3 = U.rearrange("p (m c) -> p m c", m=2)
            VE.tensor_single_scalar(ws.rearrange("p (m c) -> p m c", m=2),
                                    U3[:, :, 480:484], thr, AL.is_ge)
            w_sp.append(ws)
            if t < T - 1:
                VE.scalar_tensor_tensor(out=m_vt, in0=U, scalar=thr, in1=U,
                                        op0=AL.is_lt, op1=AL.mult)
            vk_sp.append(vs)
            # keep-warm: tiny matmul chained on this step's spikes so the PE
            # HAM window never sees a fully idle interval during the LIF scan
            nc.tensor.matmul(pwarm[:, 0:1], ident, vs[:, 0:1], start=True, stop=True)

        # ---------------- phase C: kv-first experts ----------------
        # masked spikes: w_e*(U>=thr) via one tensor_scalar (4x mode);
        # y accumulated with plain bf16 tensor_tensor adds (2x mode)
        y = [None] * T
        m_res_e = [mp.tile([P, 768], BF16, name=f"m_res{e}", tag=f"m_res{e}")
                   for e in range(NE)]
        for t in range(T):
            thr = float(2.0 ** t)
            pkv = ps_ad.tile([ED, 384], F32, name=f"pkv{t}", tag="pm")
            for mt in range(2):
                nc.tensor.matmul(pkv, vk_sp[t][:, mt*484+384:mt*484+480],
                                 vk_sp[t][:, mt*484:mt*484+384],
                                 start=(mt == 0), stop=(mt == 1))
            kv = kv_p.tile([ED, 384], BF16, name=f"kv{t}", tag="t")
            nc.scalar.activation(kv, pkv, AF.Copy, bias=0.0, scale=0.5 * thr)
            yt = y_p.tile([P, 768], BF16, name=f"y{t}", tag="t")
            y[t] = yt
            for e in range(NE):
                m_res = m_res_e[e]
                xr = xr_p.tile([P, 768], BF16, name=f"xres{e}{t}", tag="t")
                for mt in range(2):
                    pr_ = ps_ad.tile([P, 384], F32, name=f"pres{e}{t}{mt}", tag="pm")
                    nc.tensor.matmul(pr_, sp_q[t][:, e*N + mt*P: e*N + (mt+1)*P],
                                     kv, start=True, stop=True)
                    nc.scalar.activation(xr[:, mt*384:(mt+1)*384], pr_, AF.Copy)
                U = xr
                if t > 0:
                    VE.tensor_add(U, m_res, U)
                dst = yt if e == 0 else rs_p.tile([P, 768], BF16,
                                                  name=f"rsm{e}{t}", tag="t")
                for mt in range(2):
                    VE.tensor_scalar(out=dst[:, mt*384:(mt+1)*384],
                                     in0=U[:, mt*384:(mt+1)*384],
                                     scalar1=thr,
                                     scalar2=w_sp[t][:, mt*4+e:mt*4+e+1],
                                     op0=AL.is_ge, op1=AL.mult)
                if t < T - 1:
                    VE.scalar_tensor_tensor(out=m_res, in0=U, scalar=thr, in1=U,
                                            op0=AL.is_lt, op1=AL.mult)
                if e > 0:
                    VE.tensor_add(yt, yt, dst)
                nc.tensor.matmul(pwarm[:, 0:1], ident, U[:, 0:1], start=True, stop=True)

        # ---------------- phase D: transpose y, proj, LIF, residual ----------------
        ydn = [ydn_p.tile([P, 4 * N], BF16, name=f"ydn{dt}", tag="t") for dt in range(3)]
        xp_t = [xp_p.tile([P, 768], F32, name=f"xp{t}", tag="t") for t in range(T)]
        for tp in range(2):
            for t in (tp * 2, tp * 2 + 1):
                for mt in range(2):
                    for dt in range(3):
                        ptr = ps_ad.tile([P, P], BF16, name=f"ptr{t}{mt}{dt}", tag="pm")
                        nc.tensor.transpose(
                            ptr, y[t][:, mt*384 + dt*P: mt*384 + (dt+1)*P], ident)
                        nc.scalar.activation(ydn[dt][:, t*N + mt*P: t*N + (mt+1)*P],
                                             ptr, AF.Copy)
            for mt in range(3):
                pp = ps_ad.tile([P, 512], F32, name=f"pp{mt}_{tp}", tag="pm")
                for kt in range(3):
                    nc.tensor.matmul(pp, pj_w[kt][:, mt*P:(mt+1)*P],
                                     ydn[kt][:, tp*512:(tp+1)*512],
                                     start=(kt == 0), stop=(kt == 2))
                for ti in range(2):
                    t = tp * 2 + ti
                    c = mt * 4 + t
                    nc.scalar.activation(xp_t[t][:, mt*N:(mt+1)*N], pp[:, ti*N:(ti+1)*N],
                                         AF.Identity, bias=b_p[:, c:c+1], scale=a_p[:, c:c+1])
            for t in (tp * 2, tp * 2 + 1):
                thr = float(2.0 ** t)
                U = xp_t[t]
                if t > 0:
                    GE.tensor_add(U, m_p, U)
                if t < T - 1:
                    VE.scalar_tensor_tensor(out=m_p, in0=U, scalar=thr, in1=U,
                                            op0=AL.is_lt, op1=AL.mult)
                for mt in range(3):
                    # x_new overwrites xs in place (residual add)
                    VE.scalar_tensor_tensor(
                        out=xs_kt[mt][:, t*N:(t+1)*N], in0=U[:, mt*N:(mt+1)*N],
                        scalar=thr, in1=xs_kt[mt][:, t*N:(t+1)*N],
                        op0=AL.is_ge, op1=AL.add)
            # x_new -> bf16 for this time-pair (fc1 can start on tp=0)
            for kt in range(3):
                nc.scalar.activation(xq_kt[kt][:, tp*512:(tp+1)*512],
                                     xs_kt[kt][:, tp*512:(tp+1)*512], AF.Copy)

        # ---------------- phase E: MLP ----------------
        psum_ad.close()
        ps_e = ctx.enter_context(tc.tile_pool(name="ps_e", bufs=3, space="PSUM"))
        ps_dw = ctx.enter_context(tc.tile_pool(name="ps_dw", bufs=2, space="PSUM"))
        ps_po = ctx.enter_context(tc.tile_pool(name="ps_po", bufs=3, space="PSUM"))
        # padded dw-spike tiles (zero borders written once)
        xp1_bufs = [xp1_p.tile([P, 1296], BF16, name=f"xp1_{i}", tag=f"xp1_{i}")
                    for i in range(2)]
        for b_ in xp1_bufs:
            GE.memset(b_, 0.0)
        po1 = [ps_po.tile([P, 512], F32, name=f"po1_{mt}", tag="po")
               for mt in range(3)]
        TAPS = [(dy, dx) for dy in range(3) for dx in range(3)]
        mg_t = []
        for ch in range(8):
            xh = xh_p.tile([P, 2048], BF16, name=f"xh{ch}", tag="t")
            for half in range(2):
                mth = ch + 8 * half
                for tp in range(2):
                    ph = ps_e.tile([P, 512], F32, name=f"ph{ch}{half}{tp}", tag="pm")
                    for kt in range(3):
                        nc.tensor.matmul(ph, f1_w[kt][:, mth*P:(mth+1)*P],
                                         xq_kt[kt][:, tp*512:(tp+1)*512],
                                         start=(kt == 0), stop=(kt == 2))
                    for ti in range(2):
                        t = tp * 2 + ti
                        c = mth * 4 + t
                        nc.scalar.activation(
                            xh[:, half*1024 + t*N: half*1024 + (t+1)*N],
                            ph[:, ti*N:(ti+1)*N], AF.Identity,
                            bias=b_h[:, c:c+1], scale=a_h[:, c:c+1])
            # h-LIF over t; dw-half spikes written into padded tile, gate into sp2
            m_h = mh_p.tile([P, 512], BF16, name=f"m_h{ch}", tag="t")
            sp2 = sp2_p.tile([P, 1024], BF16, name=f"sp2_{ch}", tag="t")
            xp1 = xp1_bufs[ch % 2]
            xh3 = xh.rearrange("p (h q) -> p h q", h=2)
            mh3 = m_h.rearrange("p (h q) -> p h q", h=2)
            xh4 = xh.rearrange("p (a h w) -> p a h w", h=16, w=16)
            xp4 = xp1.rearrange("p (t h w) -> p t h w", t=4, h=18, w=18)
            for t in range(T):
                thr = float(2.0 ** t)
                U3 = xh3[:, :, t*N:(t+1)*N]
                if t > 0:
                    VE.tensor_add(U3, mh3, U3)
                VE.tensor_single_scalar(xp4[:, t, 1:17, 1:17], xh4[:, t], thr, AL.is_ge)
                VE.tensor_single_scalar(sp2[:, t*N:(t+1)*N],
                                        xh[:, 1024 + t*N: 1024 + (t+1)*N], thr, AL.is_ge)
                if t < T - 1:
                    VE.scalar_tensor_tensor(out=mh3, in0=U3, scalar=thr, in1=U3,
                                            op0=AL.is_lt, op1=AL.mult)
            # depthwise conv on the PE: diag(bias) @ ones + sum diag(tap) @ shifted
            acc = acc_p.tile([P, 1024], BF16, name=f"acc{ch}", tag="t")
            for tb in range(2):
                pa = ps_dw.tile([P, 512], F32, name=f"dwa{ch}{tb}", tag="pdw")
                for j, (dy, dx) in enumerate(TAPS):
                    rhs = xp4[:, tb*2:(tb+1)*2, dy:dy+16, dx:dx+16]
                    nc.tensor.matmul(pa, dwd[:, (ch*9+j)*P:(ch*9+j+1)*P], rhs,
                                     start=(j == 0), stop=(j == 8))
                nc.scalar.activation(acc[:, tb*512:(tb+1)*512], pa, AF.Identity,
                                     bias=bdw[:, ch:ch+1], scale=1.0)
            # dw-LIF + gate -> mg0 (t-pair 0) / mg1 (t-pair 1), bf16
            m_dw = mdw_p.tile([P, N], BF16, name=f"m_dw{ch}", tag="t")
            mg0 = mg0_p.tile([P, 512], BF16, name=f"mg0_{ch}", tag="t")
            mg1 = mg1_p.tile([P, 512], BF16, name=f"mg1_{ch}", tag="t")
            for t in range(T):
                thr = float(2.0 ** t)
                mg = mg0 if t < 2 else mg1
                U = acc[:, t*N:(t+1)*N]
                if t > 0:
                    VE.scalar_tensor_tensor(out=U, in0=U, scalar=thr, in1=m_dw,
                                            op0=AL.mult, op1=AL.add)
                VE.scalar_tensor_tensor(out=mg[:, (t % 2)*N:(t % 2 + 1)*N], in0=U,
                                        scalar=thr, in1=sp2[:, t*N:(t+1)*N],
                                        op0=AL.is_ge, op1=AL.mult)
                if t < T - 1:
                    VE.scalar_tensor_tensor(out=m_dw, in0=U, scalar=thr, in1=U,
                                            op0=AL.is_lt, op1=AL.mult)
            mg_t.append(mg1)
            # fc2 t-pair 0 accumulate
            for mt in range(3):
                nc.tensor.matmul(po1[mt], f2_w[ch][:, mt*P:(mt+1)*P],
                                 mg0,
                                 start=(ch == 0), stop=(ch == 7), skip_group_check=True)

        # fc2 t-pair 1 tail
        po2 = [ps_po.tile([P, 512], F32, name=f"po2_{mt}", tag="po")
               for mt in range(3)]
        for mt in range(3):
            for ch in range(8):
                nc.tensor.matmul(po2[mt], f2_w[ch][:, mt*P:(mt+1)*P],
                                 mg_t[ch],
                                 start=(ch == 0), stop=(ch == 7), skip_group_check=True)

        # fc2 evict + final LIF + residual + store
        xo_t = [xo_p.tile([P, 768], F32, name=f"xo{t}", tag="t") for t in range(T)]
        for t in range(T):
            po = po1 if t < 2 else po2
            for mt in range(3):
                c = mt * 4 + t
                nc.scalar.activation(xo_t[t][:, mt*N:(mt+1)*N],
                                     po[mt][:, (t % 2)*N:(t % 2 + 1)*N],
                                     AF.Identity, bias=b_o[:, c:c+1], scale=a_o[:, c:c+1])
        for t in range(T):
            thr = float(2.0 ** t)
            for mt in range(3):
                U = xo_t[t][:, mt*N:(mt+1)*N]
                if t > 0:
                    GE.tensor_add(U, m_o[mt], U)
                if t < T - 1:
                    VE.scalar_tensor_tensor(out=m_o[mt], in0=U, scalar=thr, in1=U,
                                            op0=AL.is_lt, op1=AL.mult)
                # final out in place over xo (reset already consumed U)
                VE.scalar_tensor_tensor(
                    out=U, in0=U, scalar=thr,
                    in1=xs_kt[mt][:, t*N:(t+1)*N], op0=AL.is_ge, op1=AL.add)
                nc.sync.dma_start(out=d['out'][t*C + mt*P: t*C + (mt+1)*P, :],
                                  in_=U)


def _build():
    nc = bacc.Bacc()
    with tile.TileContext(nc) as tc:
        with tc.tile_pool(name="dram", bufs=1, space="DRAM") as dram:
            def din(name, shape, dt=F32):
                return dram.tile(shape, dt, kind="ExternalInput", name=name,
                                 uniquify=False)
            d = {
                'xin': din('xin', [C, 4 * N]),
                'out': dram.tile([T * C, N], F32, kind="ExternalOutput",
                                 name='out', uniquify=False),
                'q_w': din('q_w', [384, 384], BF16),
                's_q': din('s_q', [ED, 32]),
                'vk_w': din('vk_w', [384, 484], BF16),
                'rb2': din('rb2', [1, 484], BF16),
                'ones': din('ones', [1, 128], BF16),
                'pj_w': din('pj_w', [384, 384], BF16),
                's_po': din('s_po', [128, 48]),
                'f1_w': din('f1_w', [384, 2048], BF16),
                's_h': din('s_h', [128, 128]),
                'dwd': din('dwd', [128, 72 * 128], BF16),
                'b_dw': din('b_dw', [128, 8]),
                'f2_w': din('f2_w', [1024, 384], BF16),
                'ident': din('ident', [128, 128], BF16),
            }
            _body(nc, tc, d)
    nc.finalize()
    return nc


_NC_CACHE = {}


def _get_nc():
    if 'nc' not in _NC_CACHE:
        _NC_CACHE['nc'] = _build()
    return _NC_CACHE['nc']


def _tcols(a):
    rows, k = a.shape
    out = np.empty((rows, k * 4), np.float32)
    for u in range(k):
        for t in range(4):
            out[:, u * 4 + t] = a[:, u] * (2.0 ** t)
    return out


def _prep_common(inputs):
    inp = {k: np.asarray(v, np.float32) for k, v in inputs.items()}
    bf16 = ml_dtypes.bfloat16

    exp_wT = np.concatenate([inp['exp_w'][e].T for e in range(NE)], axis=1)
    a_q = np.zeros((ED, NE), np.float32)
    b_q = np.zeros((ED, NE), np.float32)
    for e in range(NE):
        a_q[:, e] = 0.5 * inp['exp_g'][e] * S
        b_q[:, e] = 0.5 * inp['exp_b'][e]
    rw = inp['router_w'].T * (inp['router_g'] * S)[None, :]
    vk = np.concatenate([inp['v_w'].T, inp['k_w'].T, rw], axis=1)
    rb2 = np.zeros((1, 484), np.float32)
    rb2[0, 480:484] = (inp['router_b'] * inp['router_g'] * S + inp['router_be'])

    g = inp['dw_g']
    taps = (inp['dw_w'][:, 0] * (0.5 * g * S)[:, None, None]).reshape(HH, 9)
    bias = 0.5 * (inp['dw_b'] * g * S + inp['dw_be'])
    dwd = np.zeros((P, 72, P), np.float32)
    pi = np.arange(P)
    for ch in range(8):
        cg = ch * P + pi
        for j in range(9):
            dwd[pi, ch*9+j, pi] = taps[cg, j]

    com = {
        'q_w': exp_wT.astype(bf16),
        's_q': np.concatenate([_tcols(a_q), _tcols(b_q)], axis=1),
        'vk_w': vk.astype(bf16),
        'rb2': rb2.astype(bf16),
        'ones': np.ones((1, 128), bf16),
        'pj_w': inp['proj_w'].T.astype(bf16),
        's_po': np.concatenate([
            _tcols((0.5 * inp['proj_g'] * S).reshape(3, 128).T),
            _tcols((0.5 * (inp['proj_b'] * inp['proj_g'] * S
                           + inp['proj_be'])).reshape(3, 128).T),
            _tcols((0.5 * inp['fc2_g'] * S).reshape(3, 128).T),
            _tcols((0.5 * (inp['fc2_b'] * inp['fc2_g'] * S
                           + inp['fc2_be'])).reshape(3, 128).T)], axis=1),
        'f1_w': inp['fc1_w'].T.astype(bf16),
        's_h': np.concatenate([
            _tcols((0.5 * inp['fc1_g'] * S).reshape(16, 128).T),
            _tcols((0.5 * (inp['fc1_b'] * inp['fc1_g'] * S
                           + inp['fc1_be'])).reshape(16, 128).T)], axis=1),
        'dwd': dwd.reshape(P, 72 * P).astype(bf16),
        'b_dw': np.ascontiguousarray(bias.reshape(8, P).T),
        'f2_w': inp['fc2_w'].T.astype(bf16),
        'ident': np.eye(128, dtype=bf16),
    }
    return {k: np.ascontiguousarray(v) for k, v in com.items()}


def run(inputs, trace=False, tmpdir=None):
    com = _prep_common(inputs)
    x = np.asarray(inputs['x'], np.float32).reshape(T, B, C, N)
    in_maps = []
    for b in range(B):
        m = dict(com)
        m['xin'] = np.ascontiguousarray(x[:, b].transpose(1, 0, 2).reshape(C, T * N))
        in_maps.append(m)
    res = run_bass_kernel_spmd(_get_nc(), in_maps, list(range(B)),
                               trace=trace, tmpdir=tmpdir)
    out = np.empty((T, B, C, N), np.float32)
    for b in range(B):
        out[:, b] = res.results[b]['out'].reshape(T, C, N)
    return out.reshape(T * B, C, 16, 16), res.exec_time_ns


def kernel(**inputs):
    out, _ = run(inputs)
    return out


# revision 21
# speedup vs baseline: 2.0640x; 1.1219x over previous
"""Trainium2 Bass kernel for nn_Block_31954556682442 (spiking MoE-SSA block).

Sharding: pure data-parallel over batch B=8 -> one sample (4 LIF time steps)
per NeuronCore, zero collectives. v3 design:
  - ALL weight matmuls single-term bf16 (W and x both bf16-rounded); CPU-sim
    rel err 4.4e-3 vs the 2e-2 gate
  - kv-first attention: res_e = q_e^T (k v^T); kv integers <=256 exact bf16;
    k^T computed in v's layout by widening the v matmul rhs (no transposes)
  - attention-path LIF state kept in bf16 (values quantized to 0.5 and
    compare-safe), enabling 2x/4x DVE modes
  - depthwise 3x3 conv moved to the PE: per-(ch,tap) diagonal bf16 weights
    x zero-padded spike tiles; bias via diag @ ones
  - fc2 split: t-pair 0 accumulated across the ch loop (3 PSUM banks),
    t-pair 1 as a tail overlapped with the final LIF
  - LIF scans in 2^t-scaled form as in v2
Self-contained: hardcodes all shapes; no sibling imports.
"""
import numpy as np
import ml_dtypes

import concourse.bacc as bacc
import concourse.mybir as mybir
import concourse.tile as tile
from concourse.bass_utils import run_bass_kernel_spmd

F32 = mybir.dt.float32
BF16 = mybir.dt.bfloat16
AL = mybir.AluOpType
AF = mybir.ActivationFunctionType

T, B, C, N = 4, 8, 384, 256
ED = 96
NE = 4
HID, HH = 2048, 1024
S = float(1.0 / np.sqrt(1.0 + 1e-5))
P = 128


def _body(nc, tc, d):
    from contextlib import ExitStack
    VE = nc.vector
    GE = nc.gpsimd

    with ExitStack() as ctx:
        def pool(name, bufs, space="SBUF"):
            return ctx.enter_context(tc.tile_pool(name=name, bufs=bufs, space=space))

        wp = pool("wp", 1)
        mp = pool("mp", 1)
        xs_p = pool("xs_p", 3)       # (128,1024) f32 x, doubles as x_new
        xq_p = pool("xq_p", 3)       # (128,1024) bf16 rounded x
        xqt_p = pool("xqt_p", 3)     # (96,1024) f32 q pre-act
        xvt_p = pool("xvt_p", 3)     # (128,960) f32 v|k pre-act
        xrt_p = pool("xrt_p", 2)     # (128,8)
        spq_p = pool("spq_p", 4)     # (96,1024) bf16 q spikes
        vks_p = pool("vks_p", 4)     # (128,960) bf16 v|k spikes
        wsp_p = pool("wsp_p", 4)     # (128,8) f32 router spikes
        kv_p = pool("kv_p", 2)       # (96,384) bf16
        xr_p = pool("xr_p", 3)       # (128,768) bf16 res pre-act
        rs_p = pool("rs_p", 2)       # (128,768) bf16 masked res spikes
        y_p = pool("y_p", 4)         # (128,768) bf16
        ydn_p = pool("ydn_p", 3)     # (128,1024) bf16
        xp_p = pool("xp_p", 2)       # (128,768) f32 proj pre-act
        xh_p = pool("xh_p", 2)       # (128,2048) f32
        sp2_p = pool("sp2_p", 2)     # (128,1024) bf16 gate spikes
        xp1_p = pool("xp1_p", 1)     # padded dw spikes, 2 tiles
        acc_p = pool("acc_p", 2)     # (128,1024) f32 dw-conv acc
        mg0_p = pool("mg0_p", 2)     # (128,512) bf16 t-pair0 gated spikes
        mg1_p = pool("mg1_p", 8)     # (128,512) bf16 t-pair1 gated spikes
        mh_p = pool("mh_p", 2)       # (128,512) f32
        mdw_p = pool("mdw_p", 1)     # (128,256) f32
        xo_p = pool("xo_p", 2)       # (128,768) bf16 fc2 pre-act
        of_p = pool("of_p", 2)       # (128,768) f32 final out

        psum_ad = ExitStack()
        ps_ad = psum_ad.enter_context(
            tc.tile_pool(name="ps_ad", bufs=6, space="PSUM"))

        # ---------------- weight loads ----------------
        def wload(name, shape, dt=F32, src=None):
            w = wp.tile(shape, dt, name=name, tag=name)
            nc.sync.dma_start(out=w, in_=d[name] if src is None else src)
            return w

        ident = wload('ident', [P, P], BF16)
        # PE warmup: dummy matmuls to flip HAM to K=8/8 before phase A
        pwarm = ps_ad.tile([P, P], F32, name="pwarm", tag="pm")
        for wi in range(24):
            nc.tensor.matmul(pwarm, ident, ident, start=True, stop=True)
        warm_sink = wp.tile([P, 1], F32, name="warm_sink", tag="warm_sink")
        nc.scalar.activation(warm_sink, pwarm[:, 0:1], AF.Copy)

        # xs first (phase A starts on these)
        xs_kt = []
        for kt in range(3):
            x_ = xs_p.tile([P, 4 * N], F32, name=f"xs{kt}", tag="t")
            xs_kt.append(x_)
        for kt in range(3):
            nc.sync.dma_start(out=xs_kt[kt], in_=d['xin'][kt*P:(kt+1)*P, :])
        q_w, vk_w, pj_w, f1_w, f2_w = [], [], [], [], []
        for kt in range(3):
            q_w.append(wload(f'q_w{kt}', [P, 384], BF16, d['q_w'][kt*P:(kt+1)*P, :]))
            vk_w.append(wload(f'vk_w{kt}', [P, 484], BF16, d['vk_w'][kt*P:(kt+1)*P, :]))
        sq = wload('s_q', [ED, 32])
        a_q, b_q = sq[:, 0:16], sq[:, 16:32]
        rb2 = wload('rb2', [1, 484], BF16); ones = wload('ones', [1, P], BF16)
        for kt in range(3):
            pj_w.append(wload(f'pj_w{kt}', [P, 384], BF16, d['pj_w'][kt*P:(kt+1)*P, :]))
        spo = wload('s_po', [P, 48])
        a_p, b_p = spo[:, 0:12], spo[:, 12:24]
        a_o, b_o = spo[:, 24:36], spo[:, 36:48]
        for kt in range(3):
            f1_w.append(wload(f'f1_w{kt}', [P, 2048], BF16, d['f1_w'][kt*P:(kt+1)*P, :]))
        sh = wload('s_h', [P, 128])
        a_h, b_h = sh[:, 0:64], sh[:, 64:128]
        dwd = wload('dwd', [P, 72 * P], BF16)
        bdw = wload('b_dw', [P, 8])
        for ch in range(8):
            f2_w.append(wload(f'f2_w{ch}', [P, 384], BF16, d['f2_w'][ch*P:(ch+1)*P, :]))

        # ---------------- x -> bf16 ----------------
        xq_kt = []
        for kt in range(3):
            q_ = xq_p.tile([P, 4 * N], BF16, name=f"xq{kt}", tag="t")
            nc.scalar.activation(q_, xs_kt[kt], AF.Copy)
            xq_kt.append(q_)

        # ---------------- phase A: q / v|k / router matmuls + evicts ----------------
        m_kq = mp.tile([ED, 4 * N], BF16, name="m_kq", tag="m_kq")
        m_vt = mp.tile([P, 968], BF16, name="m_vt", tag="m_vt")
        m_p = mp.tile([P, 768], BF16, name="m_p", tag="m_p")
        m_o = [mp.tile([P, N], BF16, name=f"m_o{i}", tag=f"m_o{i}") for i in range(3)]

        xq_t = [xqt_p.tile([ED, 4 * N], BF16, name=f"xqt{t}", tag="t") for t in range(T)]
        xvt_t = [xvt_p.tile([P, 968], BF16, name=f"xvt{t}", tag="t") for t in range(T)]

        for tp in range(2):
            for u in range(NE):
                pt = ps_ad.tile([ED, 512], F32, name=f"pq{u}_{tp}", tag="pm")
                for kt in range(3):
                    nc.tensor.matmul(pt, q_w[kt][:, 96*u:96*(u+1)],
                                     xq_kt[kt][:, tp*512:(tp+1)*512],
                                     start=(kt == 0), stop=(kt == 2))
                for ti in range(2):
                    t = tp * 2 + ti
                    c = u * 4 + t
                    nc.scalar.activation(xq_t[t][:, u*N:(u+1)*N], pt[:, ti*N:(ti+1)*N],
                                         AF.Identity, bias=b_q[:, c:c+1], scale=a_q[:, c:c+1])
        for t in range(T):
            for mt in range(2):
                pv = ps_ad.tile([P, 484], F32, name=f"pvt{t}_{mt}", tag="pm")
                for kt in range(3):
                    nc.tensor.matmul(pv, xq_kt[kt][:, t*N + mt*P: t*N + (mt+1)*P],
                                     vk_w[kt], start=(kt == 0), stop=False,
                                     skip_group_check=True)
                nc.tensor.matmul(pv[:, 480:484], ones, rb2[:, 480:484],
                                 start=False, stop=True, skip_group_check=True)
                nc.scalar.activation(xvt_t[t][:, mt*484:(mt+1)*484], pv, AF.Copy,
                                     bias=0.0, scale=0.5 * float(2.0 ** t))

        # ---------------- phase B: LIF scans for q / v|k / r ----------------
        sp_q, vk_sp, w_sp = [], [], []
        for t in range(T):
            thr = float(2.0 ** t)
            U = xq_t[t]
            if t > 0:
                VE.tensor_add(U, m_kq, U)
            sp = spq_p.tile([ED, 4 * N], BF16, name=f"spq{t}", tag="t")
            VE.tensor_single_scalar(sp, U, thr, AL.is_ge)
            if t < T - 1:
                VE.scalar_tensor_tensor(out=m_kq, in0=U, scalar=thr, in1=U,
                                        op0=AL.is_lt, op1=AL.mult)
            sp_q.append(sp)

            U = xvt_t[t]
            if t > 0:
                VE.tensor_add(U, m_vt, U)
            vs = vks_p.tile([P, 968], BF16, name=f"vks{t}", tag="t")
            VE.tensor_single_scalar(vs, U, thr, AL.is_ge)
            # router spike columns as f32 (tensor_scalar scalar2 operand)
            ws = wsp_p.tile([P, 8], F32, name=f"wsp{t}", tag="t")
            U3 = U.rearrange("p (m c) -> p m c", m=2)
            VE.tensor_single_scalar(ws.rearrange("p (m c) -> p m c", m=2),
                                    U3[:, :, 480:484], thr, AL.is_ge)
            w_sp.append(ws)
            if t < T - 1:
                VE.scalar_tensor_tensor(out=m_vt, in0=U, scalar=thr, in1=U,
                                        op0=AL.is_lt, op1=AL.mult)
            vk_sp.append(vs)
            # keep-warm: tiny matmul chained on this step's spikes so the PE
            # HAM window never sees a fully idle interval during the LIF scan
            nc.tensor.matmul(pwarm[:, 0:1], ident, vs[:, 0:1], start=True, stop=True)

        # ---------------- phase C: kv-first experts ----------------
        # masked spikes: w_e*(U>=thr) via one tensor_scalar (4x mode);
        # y accumulated with plain bf16 tensor_tensor adds (2x mode)
        y = [None] * T
        m_res_e = [mp.tile([P, 768], BF16, name=f"m_res{e}", tag=f"m_res{e}")
                   for e in range(NE)]
        for t in range(T):
            thr = float(2.0 ** t)
            pkv = ps_ad.tile([ED, 384], F32, name=f"pkv{t}", tag="pm")
            for mt in range(2):
                nc.tensor.matmul(pkv, vk_sp[t][:, mt*484+384:mt*484+480],
                                 vk_sp[t][:, mt*484:mt*484+384],
                                 start=(mt == 0), stop=(mt == 1))
            kv = kv_p.tile([ED, 384], BF16, name=f"kv{t}", tag="t")
            nc.scalar.activation(kv, pkv, AF.Copy, bias=0.0, scale=0.5 * thr)
            yt = y_p.tile([P, 768], BF16, name=f"y{t}", tag="t")
            y[t] = yt
            for e in range(NE):
                m_res = m_res_e[e]
                xr = xr_p.tile([P, 768], BF16, name=f"xres{e}{t}", tag="t")
                for mt in range(2):
                    pr_ = ps_ad.tile([P, 384], F32, name=f"pres{e}{t}{mt}", tag="pm")
                    nc.tensor.matmul(pr_, sp_q[t][:, e*N + mt*P: e*N + (mt+1)*P],
                                     kv, start=True, stop=True)
                    nc.scalar.activation(xr[:, mt*384:(mt+1)*384], pr_, AF.Copy)
                U = xr
                if t > 0:
                    VE.tensor_add(U, m_res, U)
                dst = yt if e == 0 else rs_p.tile([P, 768], BF16,
                                                  name=f"rsm{e}{t}", tag="t")
                for mt in range(2):
                    VE.tensor_scalar(out=dst[:, mt*384:(mt+1)*384],
                                     in0=U[:, mt*384:(mt+1)*384],
                                     scalar1=thr,
                                     scalar2=w_sp[t][:, mt*4+e:mt*4+e+1],
                                     op0=AL.is_ge, op1=AL.mult)
                if t < T - 1:
                    VE.scalar_tensor_tensor(out=m_res, in0=U, scalar=thr, in1=U,
                                            op0=AL.is_lt, op1=AL.mult)
                if e > 0:
                    VE.tensor_add(yt, yt, dst)
                nc.tensor.matmul(pwarm[:, 0:1], ident, U[:, 0:1], start=True, stop=True)

        # ---------------- phase D: transpose y, proj, LIF, residual ----------------
        ydn = [ydn_p.tile([P, 4 * N], BF16, name=f"ydn{dt}", tag="t") for dt in range(3)]
        xp_t = [xp_p.tile([P, 768], BF16, name=f"xp{t}", tag="t") for t in range(T)]
        for tp in range(2):
            for t in (tp * 2, tp * 2 + 1):
                for mt in range(2):
                    for dt in range(3):
                        ptr = ps_ad.tile([P, P], BF16, name=f"ptr{t}{mt}{dt}", tag="pm")
                        nc.tensor.transpose(
                            ptr, y[t][:, mt*384 + dt*P: mt*384 + (dt+1)*P], ident)
                        nc.scalar.activation(ydn[dt][:, t*N + mt*P: t*N + (mt+1)*P],
                                             ptr, AF.Copy)
            for mt in range(3):
                pp = ps_ad.tile([P, 512], F32, name=f"pp{mt}_{tp}", tag="pm")
                for kt in range(3):
                    nc.tensor.matmul(pp, pj_w[kt][:, mt*P:(mt+1)*P],
                                     ydn[kt][:, tp*512:(tp+1)*512],
                                     start=(kt == 0), stop=(kt == 2))
                for ti in range(2):
                    t = tp * 2 + ti
                    c = mt * 4 + t
                    nc.scalar.activation(xp_t[t][:, mt*N:(mt+1)*N], pp[:, ti*N:(ti+1)*N],
                                         AF.Identity, bias=b_p[:, c:c+1], scale=a_p[:, c:c+1])
            for t in (tp * 2, tp * 2 + 1):
                thr = float(2.0 ** t)
                U = xp_t[t]
                if t > 0:
                    VE.tensor_add(U, m_p, U)
                if t < T - 1:
                    VE.scalar_tensor_tensor(out=m_p, in0=U, scalar=thr, in1=U,
                                            op0=AL.is_lt, op1=AL.mult)
                for mt in range(3):
                    # x_new overwrites xs in place (residual add)
                    VE.scalar_tensor_tensor(
                        out=xs_kt[mt][:, t*N:(t+1)*N], in0=U[:, mt*N:(mt+1)*N],
                        scalar=thr, in1=xs_kt[mt][:, t*N:(t+1)*N],
                        op0=AL.is_ge, op1=AL.add)
            # x_new -> bf16 for this time-pair (fc1 can start on tp=0)
            for kt in range(3):
                nc.scalar.activation(xq_kt[kt][:, tp*512:(tp+1)*512],
                                     xs_kt[kt][:, tp*512:(tp+1)*512], AF.Copy)

        # ---------------- phase E: MLP ----------------
        psum_ad.close()
        ps_e = ctx.enter_context(tc.tile_pool(name="ps_e", bufs=3, space="PSUM"))
        ps_dw = ctx.enter_context(tc.tile_pool(name="ps_dw", bufs=2, space="PSUM"))
        ps_po = ctx.enter_context(tc.tile_pool(name="ps_po", bufs=3, space="PSUM"))
        # padded dw-spike tiles (zero borders written once)
        xp1_bufs = [xp1_p.tile([P, 1296], BF16, name=f"xp1_{i}", tag=f"xp1_{i}")
                    for i in range(2)]
        for b_ in xp1_bufs:
            GE.memset(b_, 0.0)
        po1 = [ps_po.tile([P, 512], F32, name=f"po1_{mt}", tag="po")
               for mt in range(3)]
        TAPS = [(dy, dx) for dy in range(3) for dx in range(3)]
        mg_t = []
        for ch in range(8):
            xh = xh_p.tile([P, 2048], BF16, name=f"xh{ch}", tag="t")
            for half in range(2):
                mth = ch + 8 * half
                for tp in range(2):
                    ph = ps_e.tile([P, 512], F32, name=f"ph{ch}{half}{tp}", tag="pm")
                    for kt in range(3):
                        nc.tensor.matmul(ph, f1_w[kt][:, mth*P:(mth+1)*P],
                                         xq_kt[kt][:, tp*512:(tp+1)*512],
                                         start=(kt == 0), stop=(kt == 2))
                    for ti in range(2):
                        t = tp * 2 + ti
                        c = mth * 4 + t
                        nc.scalar.activation(
                            xh[:, half*1024 + t*N: half*1024 + (t+1)*N],
                            ph[:, ti*N:(ti+1)*N], AF.Identity,
                            bias=b_h[:, c:c+1], scale=a_h[:, c:c+1])
            # h-LIF over t; dw-half spikes written into padded tile, gate into sp2
            m_h = mh_p.tile([P, 512], BF16, name=f"m_h{ch}", tag="t")
            sp2 = sp2_p.tile([P, 1024], BF16, name=f"sp2_{ch}", tag="t")
            xp1 = xp1_bufs[ch % 2]
            xh3 = xh.rearrange("p (h q) -> p h q", h=2)
            mh3 = m_h.rearrange("p (h q) -> p h q", h=2)
            xh4 = xh.rearrange("p (a h w) -> p a h w", h=16, w=16)
            xp4 = xp1.rearrange("p (t h w) -> p t h w", t=4, h=18, w=18)
            for t in range(T):
                thr = float(2.0 ** t)
                U3 = xh3[:, :, t*N:(t+1)*N]
                if t > 0:
                    VE.tensor_add(U3, mh3, U3)
                VE.tensor_single_scalar(xp4[:, t, 1:17, 1:17], xh4[:, t], thr, AL.is_ge)
                VE.tensor_single_scalar(sp2[:, t*N:(t+1)*N],
                                        xh[:, 1024 + t*N: 1024 + (t+1)*N], thr, AL.is_ge)
                if t < T - 1:
                    VE.scalar_tensor_tensor(out=mh3, in0=U3, scalar=thr, in1=U3,
                                            op0=AL.is_lt, op1=AL.mult)
            # depthwise conv on the PE: diag(bias) @ ones + sum diag(tap) @ shifted
            acc = acc_p.tile([P, 1024], BF16, name=f"acc{ch}", tag="t")
            for tb in range(2):
                pa = ps_dw.tile([P, 512], F32, name=f"dwa{ch}{tb}", tag="pdw")
                for j, (dy, dx) in enumerate(TAPS):
                    rhs = xp4[:, tb*2:(tb+1)*2, dy:dy+16, dx:dx+16]
                    nc.tensor.matmul(pa, dwd[:, (ch*9+j)*P:(ch*9+j+1)*P], rhs,
                                     start=(j == 0), stop=(j == 8))
                nc.scalar.activation(acc[:, tb*512:(tb+1)*512], pa, AF.Identity,
                                     bias=bdw[:, ch:ch+1], scale=1.0)
            # dw-LIF + gate -> mg0 (t-pair 0) / mg1 (t-pair 1), bf16
            m_dw = mdw_p.tile([P, N], BF16, name=f"m_dw{ch}", tag="t")
            mg0 = mg0_p.tile([P, 512], BF16, name=f"mg0_{ch}", tag="t")
            mg1 = mg1_p.tile([P, 512], BF16, name=f"mg1_{ch}", tag="t")
            for t in range(T):
                thr = float(2.0 ** t)
                mg = mg0 if t < 2 else mg1
                U = acc[:, t*N:(t+1)*N]
                if t > 0:
                    VE.scalar_tensor_tensor(out=U, in0=U, scalar=thr, in1=m_dw,
                                            op0=AL.mult, op1=AL.add)
                VE.scalar_tensor_tensor(out=mg[:, (t % 2)*N:(t % 2 + 1)*N], in0=U,
                                        scalar=thr, in1=sp2[:, t*N:(t+1)*N],
                                        op0=AL.is_ge, op1=AL.mult)
                if t < T - 1:
                    VE.scalar_tensor_tensor(out=m_dw, in0=U, scalar=thr, in1=U,
                                            op0=AL.is_lt, op1=AL.mult)
            mg_t.append(mg1)
            # fc2 t-pair 0 accumulate
            for mt in range(3):
                nc.tensor.matmul(po1[mt], f2_w[ch][:, mt*P:(mt+1)*P],
                                 mg0,
                                 start=(ch == 0), stop=(ch == 7), skip_group_check=True)

        # fc2 t-pair 1 tail
        po2 = [ps_po.tile([P, 512], F32, name=f"po2_{mt}", tag="po")
               for mt in range(3)]
        for mt in range(3):
            for ch in range(8):
                nc.tensor.matmul(po2[mt], f2_w[ch][:, mt*P:(mt+1)*P],
                                 mg_t[ch],
                                 start=(ch == 0), stop=(ch == 7), skip_group_check=True)

        # fc2 evict + final LIF + residual + store
        xo_t = [xo_p.tile([P, 768], BF16, name=f"xo{t}", tag="t") for t in range(T)]
        for t in range(T):
            po = po1 if t < 2 else po2
            for mt in range(3):
                c = mt * 4 + t
                nc.scalar.activation(xo_t[t][:, mt*N:(mt+1)*N],
                                     po[mt][:, (t % 2)*N:(t % 2 + 1)*N],
                                     AF.Identity, bias=b_o[:, c:c+1], scale=a_o[:, c:c+1])
        for t in range(T):
            thr = float(2.0 ** t)
            of = of_p.tile([P, 768], F32, name=f"of{t}", tag="t")
            for mt in range(3):
                U = xo_t[t][:, mt*N:(mt+1)*N]
                if t > 0:
                    VE.tensor_add(U, m_o[mt], U)
                if t < T - 1:
                    VE.scalar_tensor_tensor(out=m_o[mt], in0=U, scalar=thr, in1=U,
                                            op0=AL.is_lt, op1=AL.mult)
                VE.scalar_tensor_tensor(
                    out=of[:, mt*N:(mt+1)*N], in0=U, scalar=thr,
                    in1=xs_kt[mt][:, t*N:(t+1)*N], op0=AL.is_ge, op1=AL.add)
                nc.sync.dma_start(out=d['out'][t*C + mt*P: t*C + (mt+1)*P, :],
                                  in_=of[:, mt*N:(mt+1)*N])


def _build():
    nc = bacc.Bacc()
    with tile.TileContext(nc) as tc:
        with tc.tile_pool(name="dram", bufs=1, space="DRAM") as dram:
            def din(name, shape, dt=F32):
                return dram.tile(shape, dt, kind="ExternalInput", name=name,
                                 uniquify=False)
            d = {
                'xin': din('xin', [C, 4 * N]),
                'out': dram.tile([T * C, N], F32, kind="ExternalOutput",
                                 name='out', uniquify=False),
                'q_w': din('q_w', [384, 384], BF16),
                's_q': din('s_q', [ED, 32]),
                'vk_w': din('vk_w', [384, 484], BF16),
                'rb2': din('rb2', [1, 484], BF16),
                'ones': din('ones', [1, 128], BF16),
                'pj_w': din('pj_w', [384, 384], BF16),
                's_po': din('s_po', [128, 48]),
                'f1_w': din('f1_w', [384, 2048], BF16),
                's_h': din('s_h', [128, 128]),
                'dwd': din('dwd', [128, 72 * 128], BF16),
                'b_dw': din('b_dw', [128, 8]),
                'f2_w': din('f2_w', [1024, 384], BF16),
                'ident': din('ident', [128, 128], BF16),
            }
            _body(nc, tc, d)
    nc.finalize()
    return nc


_NC_CACHE = {}


def _get_nc():
    if 'nc' not in _NC_CACHE:
        _NC_CACHE['nc'] = _build()
    return _NC_CACHE['nc']


def _tcols(a):
    rows, k = a.shape
    out = np.empty((rows, k * 4), np.float32)
    for u in range(k):
        for t in range(4):
            out[:, u * 4 + t] = a[:, u] * (2.0 ** t)
    return out


def _prep_common(inputs):
    inp = {k: np.asarray(v, np.float32) for k, v in inputs.items()}
    bf16 = ml_dtypes.bfloat16

    exp_wT = np.concatenate([inp['exp_w'][e].T for e in range(NE)], axis=1)
    a_q = np.zeros((ED, NE), np.float32)
    b_q = np.zeros((ED, NE), np.float32)
    for e in range(NE):
        a_q[:, e] = 0.5 * inp['exp_g'][e] * S
        b_q[:, e] = 0.5 * inp['exp_b'][e]
    rw = inp['router_w'].T * (inp['router_g'] * S)[None, :]
    vk = np.concatenate([inp['v_w'].T, inp['k_w'].T, rw], axis=1)
    rb2 = np.zeros((1, 484), np.float32)
    rb2[0, 480:484] = (inp['router_b'] * inp['router_g'] * S + inp['router_be'])

    g = inp['dw_g']
    taps = (inp['dw_w'][:, 0] * (0.5 * g * S)[:, None, None]).reshape(HH, 9)
    bias = 0.5 * (inp['dw_b'] * g * S + inp['dw_be'])
    dwd = np.zeros((P, 72, P), np.float32)
    pi = np.arange(P)
    for ch in range(8):
        cg = ch * P + pi
        for j in range(9):
            dwd[pi, ch*9+j, pi] = taps[cg, j]

    com = {
        'q_w': exp_wT.astype(bf16),
        's_q': np.concatenate([_tcols(a_q), _tcols(b_q)], axis=1),
        'vk_w': vk.astype(bf16),
        'rb2': rb2.astype(bf16),
        'ones': np.ones((1, 128), bf16),
        'pj_w': inp['proj_w'].T.astype(bf16),
        's_po': np.concatenate([
            _tcols((0.5 * inp['proj_g'] * S).reshape(3, 128).T),
            _tcols((0.5 * (inp['proj_b'] * inp['proj_g'] * S
                           + inp['proj_be'])).reshape(3, 128).T),
            _tcols((0.5 * inp['fc2_g'] * S).reshape(3, 128).T),
            _tcols((0.5 * (inp['fc2_b'] * inp['fc2_g'] * S
                           + inp['fc2_be'])).reshape(3, 128).T)], axis=1),
        'f1_w': inp['fc1_w'].T.astype(bf16),
        's_h': np.concatenate([
            _tcols((0.5 * inp['fc1_g'] * S).reshape(16, 128).T),
            _tcols((0.5 * (inp['fc1_b'] * inp['fc1_g'] * S
                           + inp['fc1_be'])).reshape(16, 128).T)], axis=1),
        'dwd': dwd.reshape(P, 72 * P).astype(bf16),
        'b_dw': np.ascontiguousarray(bias.reshape(8, P).T),
        'f2_w': inp['fc2_w'].T.astype(bf16),
        'ident': np.eye(128, dtype=bf16),
    }
    return {k: np.ascontiguousarray(v) for k, v in com.items()}


def run(inputs, trace=False, tmpdir=None):
    com = _prep_common(inputs)
    x = np.asarray(inputs['x'], np.float32).reshape(T, B, C, N)
    in_maps = []
    for b in range(B):
        m = dict(com)
        m['xin'] = np.ascontiguousarray(x[:, b].transpose(1, 0, 2).reshape(C, T * N))
        in_maps.append(m)
    res = run_bass_kernel_spmd(_get_nc(), in_maps, list(range(B)),
                               trace=trace, tmpdir=tmpdir)
    out = np.empty((T, B, C, N), np.float32)
    for b in range(B):
        out[:, b] = res.results[b]['out'].reshape(T, C, N)
    return out.reshape(T * B, C, 16, 16), res.exec_time_ns


def kernel(**inputs):
    out, _ = run(inputs)
    return out
